# revision 35
# baseline (speedup 1.0000x reference)
"""Trainium2 Bass kernel for nn_GAT_with_LSTM (2-layer LSTM -> 8-head GAT -> GAT out).

Sharding: node/row dimension split across 8 cores (512 rows each).

Key algebraic restructuring of the GAT attention (vs. direct
exp(leakyrelu(f1+f2)) evaluation): with z = f1_i + f2_j and slope a,
    leakyrelu(z) = max(z, a*z)  =>  e = exp(lrelu(z)) = max(exp(z), exp(a*z)).
Softmax rows are invariant to any per-row (i) factor, so divide by
exp(a*f1_i):
    e'_ij = max(u_i * V_j, D_j),   u = exp((1-a)*f1), V = exp(f2), D = exp(a*f2).
This removes every full-matrix transcendental: exp() runs only on the rank-1
factors. Per 128-column chunk the e-row-block is built one of three ways,
chosen to balance engines:
  - DVE:  t = (ub * V_j) max D_j      (dual-op tensor_scalar, bf16 4x mode)
  - Pool: same op at 1x
  - ACT:  r = relu(V_j * ub - D_j)    (per-partition scale/bias APs); the
          missing mask*D_j term is added back on the PE as
          maskT_chunk @ (D (*) wpv), exact since mask is 0/1:
          mask*max(uV,D) = mask*r + mask*D.
The mask multiply runs as two tensor_tensors (DVE chunks 0:4 at bf16 2x,
Pool chunks 4:8), and the PE accumulates numerator and denominator together
(wpv's last column is ones).

The mask arrives host-side pre-transposed and pre-cast to bf16 (adj[blk].T),
loaded at t=0 while the LSTM runs. The LSTM packs two 256-node groups into
the partition dim with block-diagonal host-packed weights (one sigmoid op
covers i+f+o), runs bf16 matmuls, and software-pipelines layer 1 one step
behind layer 0. LSTM h outputs accumulate in SBUF so 4 batched DMAs
publish g1in for the AllGather (the baseline used 32 serialized stores).
The LSTM feature AllGather ships fp8e4 (f1/f2 logit noise ~0.05 and Wh value
noise average out across the ~2048-wide attention sums).

Activation tables (Sigmoid/Tanh at t=0; Exp/Ln/Relu during the g1 gather)
are preloaded with dummy ops so table loads stay off the critical path.

Head prep (Wh + f2 factors) is pipelined at attention-group granularity:
heads 0/1 prep interleaves with their own attention groups right after the
gather lands; later heads prep 1-2 groups ahead inside the pair loop. The
output-layer projection (pwo) and its gather payload accumulate
incrementally per head-pair on the PE.

Softmax max-subtraction is skipped: attention logits are O(1) (0.1-scale
Xavier weights), exp cannot overflow, softmax is shift-invariant.
"""

import json

import numpy as np
import ml_dtypes

import bass_rust
import concourse.bass as bass
import concourse.tile as tile
from concourse import mybir
from concourse.bass_utils import run_bass_kernel_spmd
from concourse.masks import make_identity

F32 = mybir.dt.float32
BF16 = mybir.dt.bfloat16
FP8 = mybir.dt.float8e4
I32 = mybir.dt.int32
AF = mybir.ActivationFunctionType
OP = mybir.AluOpType

NCORES = 8
N = 4096
R = N // NCORES          # 512 rows per core
SEQ, NIN, LH = 8, 2, 12
FEAT = SEQ * LH          # 96
NHID, NHEADS, NCLASS = 64, 8, 16
ALPHA = 0.2
NJC = N // 128           # 32 j-chunks
NSUB = R // 128          # 4 row sub-blocks per core
GRP = 8                  # j-chunks per group
RH = R // 2              # 256-node half (LSTM partition packing)
NGRP = NJC // GRP        # 4 groups
GOUT = NCLASS + 2        # gathered g2 row: [Whout(16) | ones | f2o]

# chunk-q assignment within each GRP of 8 chunks. The wide mask-mul TT runs
# solo on DVE (one instruction at the bf16 2x rate); splitting it or moving
# tq builds onto DVE loses to per-instruction overhead.
Q_ACT = (5, 6, 7)        # tq via ACT relu (needs mask*D PE correction)
Q_DVE = ()               # remaining tq on Pool


def _split_sync_waits(nc, max_waits=1):
    """This walrus build rejects >1 sync wait per TPB_CTRL instruction
    ("Too many sync wait commands"). Move excess waits onto NoOps inserted
    just before; same-engine program order preserves the semantics."""
    m = json.loads(bass_rust.module_to_json_string(nc.m))
    ctr = 0
    for fn in m["functions"]:
        for bb in fn["blocks"]:
            out = []
            for inst in bb["instructions"]:
                si = inst.get("sync_info")
                ow = (si or {}).get("on_wait") or []
                if len(ow) > max_waits:
                    excess, keep = ow[:-max_waits], ow[-max_waits:]
                    for i in range(0, len(excess), max_waits):
                        ctr += 1
                        out.append({
                            "engine": inst["engine"], "ins": [], "outs": [],
                            "name": f"wsplit-{ctr}", "opcode": "NoOp",
                            "sync_info": {"on_update": [],
                                          "on_wait": excess[i:i + max_waits]},
                        })
                    si["on_wait"] = keep
                out.append(inst)
            bb["instructions"] = out
    nc.m = bass_rust.module_from_json_bytes(json.dumps(m).encode())


RQ = RH // 2  # 128-node quarter: free-dim half of a packed 256 pair


class _LstmState:
    def __init__(self, lay, p1, wih, whh, b, xin):
        self.lay, self.wih, self.whh, self.b, self.xin = lay, wih, whh, b, xin
        # c lives at partition base 32 so TensorTensor partners the f-gate
        # slice (walrus requires equal SBUF base partitions for both inputs)
        self.c01 = [p1.tile([56, RQ], F32, tag=f"c{lay}_{hf}",
                            name=f"c{lay}_{hf}")[32:56, :] for hf in (0, 1)]
        self.hprev = [None, None]


def _lstm_step(nc, st, t, hf, psg, lwork, hpool):
    """One packed LSTM step for free-half hf. Partition layout (two 256-node
    groups packed): i at rows 0:24, f at 32:56, o at 64:88, g at 96:120.
    One sigmoid covers i/f/o. The two free-halves are independent chains, so
    four chains (2 layers x 2 halves) pipeline across the engines."""
    lay = st.lay
    g = psg.tile([128, RQ], F32, tag=f"g{hf}", name=f"g{lay}_{t}_{hf}")
    nc.tensor.matmul(g, st.wih, st.xin(t, hf), start=True, stop=(t == 0))
    if t > 0:
        nc.tensor.matmul(g, st.whh, st.hprev[hf], start=False, stop=True)
    sfi = lwork.tile([88, RQ], F32, tag=f"sfi{hf}", name=f"sfi{lay}_{t}_{hf}")
    nc.scalar.activation(sfi, g[0:88, :], AF.Sigmoid, bias=st.b[0:88, :])
    tg = lwork.tile([24, RQ], F32, tag=f"tg{hf}", name=f"tg{lay}_{t}_{hf}")
    nc.scalar.activation(tg, g[96:120, :], AF.Tanh, bias=st.b[96:120, :])
    c01 = st.c01[hf]
    if t == 0:
        nc.gpsimd.tensor_mul(c01, sfi[0:24, :], tg)
    else:
        # ig at base 32 to partner c01; th at base 64 to partner the o slice.
        # c-mul on Pool, ig/c-add split keeps the DVE (the LSTM-phase
        # bottleneck) down to one op per step.
        ig = lwork.tile([56, RQ], F32, tag=f"ig{hf}",
                        name=f"ig{lay}_{t}_{hf}")[32:56, :]
        nc.gpsimd.tensor_mul(ig, sfi[0:24, :], tg)
        nc.gpsimd.tensor_mul(c01, sfi[32:56, :], c01)
        nc.vector.tensor_add(c01, c01, ig)
    th = lwork.tile([88, RQ], F32, tag=f"th{hf}",
                    name=f"th{lay}_{t}_{hf}")[64:88, :]
    nc.scalar.activation(th, c01, AF.Tanh)
    h = hpool.tile([24, RQ], BF16, tag=f"h{lay}_{hf}", name=f"h{lay}_{t}_{hf}")
    nc.vector.tensor_mul(h, sfi[64:88, :], th)
    st.hprev[hf] = h
    return h


def _attention_group(nc, awork, pv, cg, ub, vcols, dcols, negd, wpv, wpv_d,
                     maskT, pfx):
    """One GRP-chunk group of the masked-softmax attention accumulation."""
    tq = awork.tile([128, GRP, R], BF16, tag="tq", name=f"tq_{pfx}_{cg}")
    for q in range(GRP):
        c = cg * GRP + q
        if q in Q_ACT:
            nc.scalar.activation(tq[:, q, :], ub, AF.Relu,
                                 scale=vcols[:, c, :],
                                 bias=negd[:, c, :])
        else:
            eng = nc.vector if q in Q_DVE else nc.gpsimd
            eng.tensor_scalar(tq[:, q, :], ub, scalar1=vcols[:, c, :],
                              scalar2=dcols[:, c, :],
                              op0=OP.mult, op1=OP.max)
    e3 = awork.tile([128, GRP, R], BF16, tag="e3", name=f"e3_{pfx}_{cg}")
    nc.vector.tensor_mul(e3, tq, maskT[:, cg * GRP:(cg + 1) * GRP, :])
    last = cg == NGRP - 1
    for q in range(GRP):
        c = cg * GRP + q
        for s in range(NSUB):
            nc.tensor.matmul(pv[:, s, :], e3[:, q, 128 * s:128 * (s + 1)],
                             wpv[:, c, :], start=(c == 0),
                             stop=(last and q == GRP - 1))
        if q in Q_ACT:
            # mask*D_j correction for the relu form
            for s in range(NSUB):
                nc.tensor.matmul(pv[:, s, :],
                                 maskT[:, c, 128 * s:128 * (s + 1)],
                                 wpv_d[:, cg * len(Q_ACT) + Q_ACT.index(q), :],
                                 start=False, stop=False)


def _elu_into(nc, awork, dst, z, pfx):
    """dst = elu(z) = min(exp(z),1)-1 + max(z,0), elementwise."""
    ez = awork.tile(list(z.shape), F32, tag="elu_ez", name=f"ez_{pfx}")
    nc.scalar.activation(ez, z, AF.Exp)
    nc.gpsimd.tensor_scalar(ez, ez, scalar1=1.0, scalar2=-1.0,
                            op0=OP.min, op1=OP.add)
    zr = awork.tile(list(z.shape), F32, tag="elu_zr", name=f"zr_{pfx}")
    nc.gpsimd.tensor_scalar(zr, z, scalar1=0.0, scalar2=None, op0=OP.max)
    nc.gpsimd.tensor_add(dst, ez, zr)


def _ubcast(nc, psf1, awork, ubpool, ones1, coefT, feats, nk, scale, pfx):
    """u = exp(scale * (coefT.T @ feats)) broadcast over partitions."""
    pf1 = psf1.tile([1, R], F32, tag="f1r", name=f"pf1_{pfx}")
    if nk == 1:
        nc.tensor.matmul(pf1, coefT, feats, start=True, stop=True)
    else:
        for fc in range(nk):
            nc.tensor.matmul(pf1, coefT[:, fc, :], feats[:, fc, :],
                             start=(fc == 0), stop=(fc == nk - 1))
    f1row = awork.tile([1, R], BF16, tag="f1row", name=f"f1row_{pfx}")
    nc.scalar.copy(f1row, pf1)
    pf1b = psf1.tile([128, R], F32, tag="f1r", name=f"pf1b_{pfx}")
    nc.tensor.matmul(pf1b, ones1, f1row, start=True, stop=True)
    ub = ubpool.tile([128, R], BF16, tag=f"ub_{pfx}", name=f"ub_{pfx}")
    nc.scalar.activation(ub, pf1b, AF.Exp, scale=scale)
    return ub


def _build_program():
    nc = bass.Bass()

    xp = nc.dram_tensor("xp", [2 * NIN, SEQ, RH], BF16, kind="ExternalInput")
    adjTb = nc.dram_tensor("adjTb", [N, R], BF16, kind="ExternalInput")
    lwts = nc.dram_tensor("lwts", [128, 128], F32, kind="ExternalInput")
    bds = nc.dram_tensor("bds", [128, 2], F32, kind="ExternalInput")
    wcat = nc.dram_tensor("wcat", [NHEADS, FEAT, NHID + 2], FP8,
                          kind="ExternalInput")
    wocat = nc.dram_tensor("wocat", [NHEADS * NHID, NCLASS + 2], BF16,
                           kind="ExternalInput")
    outb = nc.dram_tensor("outb", [R, NCLASS], F32, kind="ExternalOutput")

    with tile.TileContext(nc) as tc:
        with tc.tile_pool(name="cst", bufs=1) as cst, \
             tc.tile_pool(name="dram", bufs=1, space="DRAM") as dram:

            ident = cst.tile([128, 128], BF16)
            make_identity(nc, ident)
            ones1 = cst.tile([1, 128], BF16)
            nc.vector.memset(ones1, 1.0)
            maskT = cst.tile([128, NJC, R], BF16)
            hT_own = cst.tile([FEAT, R], FP8)
            # gathered features split in two tiles so group-0 prep only
            # waits on the first (smaller) load
            hT_a = cst.tile([FEAT, 2, R], FP8)
            hT_b = cst.tile([FEAT, NCORES - 2, R], FP8)
            wcsb = cst.tile([FEAT, NHEADS, NHID + 2], FP8)
            wocsb = cst.tile([128, NSUB, NCLASS + 2], BF16)
            dumA = cst.tile([1, 2], F32)
            dumB = cst.tile([1, 2], F32)

            g1in = dram.tile([FEAT, R], FP8)
            g1out = dram.tile([NCORES * FEAT, R], FP8, addr_space="Shared")
            g2in = dram.tile([R, GOUT], BF16)
            g2out = dram.tile([N, GOUT], BF16, addr_space="Shared")

            # ======== Phase 1: LSTM (own nodes, 2 groups packed) ===========
            with tc.tile_pool(name="p1", bufs=1) as p1, \
                 tc.tile_pool(name="psg", bufs=4, space="PSUM") as psg, \
                 tc.tile_pool(name="hpool0", bufs=SEQ) as hpool0, \
                 tc.tile_pool(name="hpool1", bufs=3) as hpool1, \
                 tc.tile_pool(name="lwork", bufs=6) as lwork:

                xp_sb = p1.tile([2 * NIN, SEQ, RH], BF16)
                nc.sync.dma_start(out=xp_sb, in_=xp[:])
                lw = p1.tile([128, 128], F32)
                nc.sync.dma_start(out=lw, in_=lwts[:])
                bt = p1.tile([128, 2], F32)
                nc.sync.dma_start(out=bt, in_=bds[:])
                # head weights + output weights: single DMAs, early
                nc.sync.dma_start(out=wcsb,
                                  in_=wcat[:].rearrange("h f c -> f h c"))
                nc.sync.dma_start(
                    out=wocsb, in_=wocat.rearrange("(c p) f -> p c f", p=128))
                # mask loads at t=0: SP is otherwise idle during the LSTM
                adjTr = adjTb[:].rearrange("(c p) r -> p c r", p=128)
                for mg in range(4):
                    nc.sync.dma_start(out=maskT[:, 8 * mg:8 * (mg + 1), :],
                                      in_=adjTr[:, 8 * mg:8 * (mg + 1), :])

                # preload Sigmoid/Tanh activation tables during input DMAs
                nc.vector.memset(dumA, 0.0)
                nc.scalar.activation(dumB, dumA, AF.Sigmoid)
                nc.scalar.activation(dumB, dumA, AF.Tanh)

                w0 = p1.tile([2 * NIN, 128], BF16)
                w0h = p1.tile([24, 128], BF16)
                w1 = p1.tile([24, 128], BF16)
                w1h = p1.tile([24, 128], BF16)
                nc.vector.tensor_copy(w0, lw[0:2 * NIN, :])
                nc.vector.tensor_copy(w0h, lw[32:56, :])
                nc.vector.tensor_copy(w1, lw[64:88, :])
                nc.vector.tensor_copy(w1h, lw[96:120, :])
                b0 = bt[:, 0:1]
                b1 = bt[:, 1:2]

                st0 = _LstmState(0, p1, w0, w0h, b0,
                                 lambda t, hf: xp_sb[:, t, RQ * hf:RQ * (hf + 1)])
                h0s = [[], []]
                st1 = _LstmState(1, p1, w1, w1h, b1,
                                 lambda t, hf: h0s[hf][t])

                # software-pipeline: layer 1 runs one step behind layer 0;
                # the two free-halves are independent chains, so four chains
                # interleave on every engine queue. Layer-1 h accumulates in
                # SBUF (free-dim placement keeps partition bases legal) so
                # only 4 batched DMAs publish g1in at the end.
                hacc = [p1.tile([24, SEQ, RQ], FP8, name=f"hacc{hf}")
                        for hf in (0, 1)]
                # g1in[12t+l, 128*(2g+hf)+c] = hacc[hf][12g+l, t, c]
                g1v = g1in[:].rearrange("(t l) (b c) -> l t b c", l=LH, b=4)
                for slot in range(SEQ + 1):
                    for hf in (0, 1):
                        if slot < SEQ:
                            h0s[hf].append(_lstm_step(nc, st0, slot, hf, psg,
                                                      lwork, hpool0))
                    for hf in (0, 1):
                        if slot >= 1:
                            t = slot - 1
                            h1 = _lstm_step(nc, st1, t, hf, psg, lwork,
                                            hpool1)
                            nc.gpsimd.tensor_copy(hacc[hf][:, t, :], h1)
                    if slot == SEQ - 2:
                        # steps 0..5 are final: publish them while steps 6-7
                        # compute, leaving only tiny stores at the end
                        for g in (0, 1):
                            for hf in (0, 1):
                                nc.sync.dma_start(
                                    out=g1v[:, 0:SEQ - 2, 2 * g + hf, :],
                                    in_=hacc[hf][LH * g:LH * (g + 1),
                                                 0:SEQ - 2, :])
                # final 2 steps: 2 stores on SP + 2 on Act so they drain in
                # parallel right as the last h lands
                for gi, (g, hf) in enumerate(((0, 0), (0, 1), (1, 0), (1, 1))):
                    eng = nc.sync if gi < 2 else nc.scalar
                    eng.dma_start(
                        out=g1v[:, SEQ - 2:SEQ, 2 * g + hf, :],
                        in_=hacc[hf][LH * g:LH * (g + 1), SEQ - 2:SEQ, :])
                nc.gpsimd.collective_compute(
                    "AllGather", OP.bypass,
                    replica_groups=[list(range(NCORES))],
                    ins=[g1in[:].opt()],
                    outs=[g1out[:].opt()])
                nc.sync.dma_start(out=hT_own, in_=g1in)

            # ======== Phase 2: 8 GAT heads + output GAT layer ===============
            with tc.tile_pool(name="att", bufs=1) as att, \
                 tc.tile_pool(name="pstr", bufs=1, space="PSUM") as pstr, \
                 tc.tile_pool(name="pswh", bufs=3, space="PSUM") as pswh, \
                 tc.tile_pool(name="psf1", bufs=1, space="PSUM") as psf1, \
                 tc.tile_pool(name="psout", bufs=1, space="PSUM") as psout, \
                 tc.tile_pool(name="pspv", bufs=2, space="PSUM") as pspv, \
                 tc.tile_pool(name="hw", bufs=4) as hw, \
                 tc.tile_pool(name="awork", bufs=4) as awork:

                hcat = att.tile([128, NSUB, NHEADS * NHID], BF16)
                hcatT = att.tile([128, NSUB, R], BF16)

                # preload Exp/Ln/Relu tables + u for all 8 heads (overlaps
                # the g1 AllGather latency: needs only hT_own)
                nc.scalar.activation(dumB, dumA, AF.Exp)
                nc.scalar.activation(dumB, dumA, AF.Ln)
                nc.scalar.activation(dumB, dumA, AF.Relu)
                ubs = []
                for h in range(NHEADS):
                    ubs.append(_ubcast(nc, psf1, awork, att, ones1,
                                       wcsb[:, h, NHID:NHID + 1], hT_own, 1,
                                       1.0 - ALPHA, f"h{h}"))

                # gathered features: 2 DMAs so head-0 prep starts early
                g1r = g1out[:].rearrange("(b f) r -> f b r", f=FEAT)
                nc.sync.dma_start(out=hT_a, in_=g1r[:, 0:2, :])
                nc.sync.dma_start(out=hT_b, in_=g1r[:, 2:NCORES, :])

                def _head_prep_start(h):
                    whpv = hw.tile([128, NJC, NHID + 1], BF16, tag="whpv",
                                   name=f"whpv{h}")
                    nc.vector.memset(whpv[:, :, NHID:NHID + 1], 1.0)
                    f2cols = hw.tile([128, NJC, 1], F32, tag="f2cols",
                                     name=f"f2cols{h}")
                    vcols = hw.tile([128, NJC, 1], F32, tag="vcols",
                                    name=f"vcols{h}")
                    dcols = hw.tile([128, NJC, 1], F32, tag="dcols",
                                    name=f"dcols{h}")
                    negd = hw.tile([128, NJC, 1], F32, tag="negd",
                                   name=f"negd{h}")
                    wpvd = hw.tile([128, NGRP * len(Q_ACT), NHID + 1], BF16,
                                   tag="wpvd", name=f"wpvd{h}")
                    return [whpv, f2cols, vcols, dcols, negd, wpvd, None,
                            None]

                def _head_prep_piece(h, st, gq):
                    # Wh (+f2) for 4 chunks; small pieces keep the in-order
                    # ACT queue smooth so attention relu-P1s are not stalled
                    # behind a prep burst
                    whpv, f2cols = st[0], st[1]
                    pw4 = pswh.tile([128, 4, NHID + 2], F32, tag="wh",
                                    name=f"pw{h}_{gq}")
                    for k in range(4):
                        c = 4 * gq + k
                        blk, co = c // 4, 128 * (c % 4)
                        src = (hT_a[:, blk, co:co + 128] if blk < 2
                               else hT_b[:, blk - 2, co:co + 128])
                        nc.tensor.matmul(pw4[:, k, :], src,
                                         wcsb[:, h, :], start=True, stop=True)
                    nc.scalar.copy(whpv[:, 4 * gq:4 * (gq + 1), 0:NHID],
                                   pw4[:, :, 0:NHID])
                    nc.scalar.copy(f2cols[:, 4 * gq:4 * (gq + 1), :],
                                   pw4[:, :, NHID + 1:NHID + 2])

                def _factors_group(h, st, cg):
                    # V/D/negD for chunk-group cg (pieces 2cg, 2cg+1); the
                    # wpvd scalings are emitted just-in-time at the consuming
                    # attention group so they never head-of-line block DVE
                    whpv, f2cols, vcols, dcols, negd, wpvd = st[:6]
                    cs = slice(GRP * cg, GRP * (cg + 1))
                    nc.scalar.activation(vcols[:, cs, :], f2cols[:, cs, :],
                                         AF.Exp)
                    nc.scalar.activation(dcols[:, cs, :], f2cols[:, cs, :],
                                         AF.Exp, scale=ALPHA)
                    nc.gpsimd.tensor_scalar(negd[:, cs, :], dcols[:, cs, :],
                                            scalar1=-1.0, scalar2=None,
                                            op0=OP.mult)

                def _wpvd_jit(st, cg):
                    whpv, _, _, dcols, _, wpvd = st[:6]
                    for qi, q in enumerate(Q_ACT):
                        c = cg * GRP + q
                        nc.vector.tensor_scalar_mul(
                            wpvd[:, cg * len(Q_ACT) + qi, :],
                            whpv[:, c, :], dcols[:, c, :])

                def _prep_task(h, st, j):
                    _head_prep_piece(h, st, 2 * j)
                    _head_prep_piece(h, st, 2 * j + 1)
                    _factors_group(h, st, j)

                def _prep_pe(h, st, j):
                    # PE half of a prep task: Wh matmuls for pieces 2j, 2j+1
                    # (slots into the PE idle window under the mask-mul TT)
                    for gq in (2 * j, 2 * j + 1):
                        pw4 = pswh.tile([128, 4, NHID + 2], F32, tag="wh",
                                        name=f"pw{h}_{gq}")
                        for k in range(4):
                            c = 4 * gq + k
                            blk, co = c // 4, 128 * (c % 4)
                            src = (hT_a[:, blk, co:co + 128] if blk < 2
                                   else hT_b[:, blk - 2, co:co + 128])
                            nc.tensor.matmul(pw4[:, k, :], src,
                                             wcsb[:, h, :], start=True,
                                             stop=True)
                        st[6 + gq % 2] = pw4

                def _prep_act(h, st, j):
                    # ACT half: PSUM->SBUF copies + factor exps
                    whpv, f2cols = st[0], st[1]
                    for gq in (2 * j, 2 * j + 1):
                        pw4 = st[6 + gq % 2]
                        nc.scalar.copy(whpv[:, 4 * gq:4 * (gq + 1), 0:NHID],
                                       pw4[:, :, 0:NHID])
                        nc.scalar.copy(f2cols[:, 4 * gq:4 * (gq + 1), :],
                                       pw4[:, :, NHID + 1:NHID + 2])
                    _factors_group(h, st, j)

                def _head_post(h, pv):
                    zall = awork.tile([128, NSUB, NHID], F32, tag="zall",
                                      name=f"zall{h}")
                    for s in range(NSUB):
                        rcp = awork.tile([128, 1], F32, tag="rcp",
                                         name=f"rcp{h}_{s}")
                        nc.vector.reciprocal(rcp, pv[:, s, NHID:NHID + 1])
                        nc.vector.tensor_scalar_mul(zall[:, s, :],
                                                    pv[:, s, 0:NHID], rcp)
                    _elu_into(nc, awork, hcat[:, :, NHID * h:NHID * (h + 1)],
                              zall, f"h{h}")

                # output-layer projection: per-pair PSUM groups (a PE
                # accumulation group must not stay open across interleaved
                # transposes), summed into an SBUF accumulator
                pwacc = att.tile([128, NSUB, NCLASS + 2], F32)
                pwos = {}

                def _transpose_piece(hp, s):
                    ptr = pstr.tile([128, 128], BF16, tag="tr",
                                    name=f"trp{hp}_{s}")
                    nc.tensor.transpose(
                        ptr, hcat[:, s, 128 * hp:128 * (hp + 1)], ident)
                    nc.scalar.copy(hcatT[:, hp, 128 * s:128 * (s + 1)], ptr)
                    if hp not in pwos:
                        pwos[hp] = psout.tile([128, NSUB, NCLASS + 2], F32,
                                              tag="pwo", name=f"pwo{hp}")
                    nc.tensor.matmul(pwos[hp][:, s, :],
                                     hcatT[:, hp, 128 * s:128 * (s + 1)],
                                     wocsb[:, hp, :], start=True, stop=True)

                def _pwo_accum(hp):
                    if hp == 0:
                        nc.vector.tensor_copy(pwacc, pwos[hp])
                    else:
                        nc.vector.tensor_add(pwacc, pwacc, pwos[hp])
                    del pwos[hp]

                def _prep_for(h, j, fn):
                    if h >= NHEADS or j > 3:
                        return
                    if h not in sts:
                        sts[h] = _head_prep_start(h)
                    fn(h, sts[h], j)

                sts = {0: _head_prep_start(0), 1: _head_prep_start(1)}
                _prep_task(0, sts[0], 0)
                _prep_task(1, sts[1], 0)
                _prep_task(0, sts[0], 1)
                _prep_task(1, sts[1], 1)

                # prep emission: a full pair of lookahead. PE halves of
                # pair hp+1's tasks go BEFORE the slot's attention group
                # (the PE is idle under the mask-mul TT); ACT halves go
                # after the first head's group so they never delay the
                # current slot's relu builds. Pair 0's own heads are
                # just-in-time whole tasks.
                for hp in range(NHEADS // 2):
                    ha, hb = 2 * hp, 2 * hp + 1
                    pa = sts[ha]
                    pb = sts[hb]
                    pva = pspv.tile([128, NSUB, NHID + 1], F32, tag="pv",
                                    name=f"pv_h{ha}")
                    pvb = pspv.tile([128, NSUB, NHID + 1], F32, tag="pv",
                                    name=f"pv_h{hb}")
                    for cg in range(NGRP):
                        if hp == 0 and 1 <= cg < 3:
                            _prep_for(ha, cg + 1, _prep_task)
                        _prep_for(ha + 2, cg, _prep_pe)
                        _wpvd_jit(pa, cg)
                        _attention_group(nc, awork, pva, cg, ubs[ha], pa[2],
                                         pa[3], pa[4], pa[0], pa[5], maskT,
                                         f"h{ha}")
                        _prep_for(ha + 2, cg, _prep_act)
                        if hp == 0 and 1 <= cg < 3:
                            _prep_for(hb, cg + 1, _prep_task)
                        _prep_for(hb + 2, cg, _prep_pe)
                        _wpvd_jit(pb, cg)
                        _attention_group(nc, awork, pvb, cg, ubs[hb], pb[2],
                                         pb[3], pb[4], pb[0], pb[5], maskT,
                                         f"h{hb}")
                        _prep_for(hb + 2, cg, _prep_act)
                        if hp >= 1:
                            # deferred: previous pair's hcatT transpose +
                            # output-projection piece (one sub-block per
                            # slot). Deferring keeps these off the PE queue
                            # head at the pair boundary, where they would
                            # stall the next pair behind the elu chain.
                            _transpose_piece(hp - 1, cg)
                    del sts[ha], sts[hb]
                    _head_post(ha, pva)
                    _head_post(hb, pvb)
                    if hp >= 1:
                        _pwo_accum(hp - 1)

                for s in range(NSUB):
                    _transpose_piece(NHEADS // 2 - 1, s)
                _pwo_accum(NHEADS // 2 - 1)

                # ---- publish output-layer gather payload ----
                # g2 row: [Whout(0:16) | ones(16) | f2o(17)]
                g2stage = awork.tile([128, NSUB, GOUT], BF16, tag="g2stage")
                nc.vector.memset(g2stage[:, :, NCLASS:NCLASS + 1], 1.0)
                nc.scalar.copy(g2stage[:, :, 0:NCLASS], pwacc[:, :, 0:NCLASS])
                nc.scalar.copy(g2stage[:, :, NCLASS + 1:NCLASS + 2],
                               pwacc[:, :, NCLASS + 1:NCLASS + 2])
                nc.sync.dma_start(
                    out=g2in[:].rearrange("(c p) f -> p c f", p=128),
                    in_=g2stage)

                ub_o = _ubcast(nc, psf1, awork, att, ones1,
                               wocsb[:, :, NCLASS:NCLASS + 1], hcatT, NSUB,
                               1.0 - ALPHA, "o")

                nc.gpsimd.collective_compute(
                    "AllGather", OP.bypass,
                    replica_groups=[list(range(NCORES))],
                    ins=[g2in[:].opt()], outs=[g2out[:].opt()])

                # ---- output attention (pipelined per group) ----
                g2full = hw.tile([128, NJC, GOUT], BF16, tag="g2full",
                                 name="g2full")
                vocols = hw.tile([128, NJC, 1], F32, tag="vcols",
                                 name="vocols")
                docols = hw.tile([128, NJC, 1], F32, tag="dcols",
                                 name="docols")
                negdo = hw.tile([128, NJC, 1], F32, tag="negd", name="negdo")
                wpvdo = hw.tile([128, NGRP * len(Q_ACT), NCLASS + 1],
                                BF16, tag="wpvd", name="wpvdo")
                pvo = pspv.tile([128, NSUB, NCLASS + 1], F32, tag="pv",
                                name="pv_o")
                g2r = g2out[:].rearrange("(c p) f -> p c f", p=128)
                for cg in range(NGRP):
                    cs = slice(GRP * cg, GRP * (cg + 1))
                    nc.sync.dma_start(out=g2full[:, cs, :], in_=g2r[:, cs, :])
                    nc.scalar.activation(vocols[:, cs, :],
                                         g2full[:, cs, NCLASS + 1:NCLASS + 2],
                                         AF.Exp)
                    nc.scalar.activation(docols[:, cs, :],
                                         g2full[:, cs, NCLASS + 1:NCLASS + 2],
                                         AF.Exp, scale=ALPHA)
                    nc.gpsimd.tensor_scalar(negdo[:, cs, :], docols[:, cs, :],
                                            scalar1=-1.0, scalar2=None,
                                            op0=OP.mult)
                    for qi, q in enumerate(Q_ACT):
                        c = cg * GRP + q
                        nc.vector.tensor_scalar_mul(
                            wpvdo[:, cg * len(Q_ACT) + qi, :],
                            g2full[:, c, 0:NCLASS + 1], docols[:, c, :])
                    _attention_group(nc, awork, pvo, cg, ub_o, vocols,
                                     docols, negdo,
                                     g2full[:, :, 0:NCLASS + 1], wpvdo,
                                     maskT, "o")
                zoall = awork.tile([128, NSUB, NCLASS], F32, tag="zoall")
                for s in range(NSUB):
                    rcp = awork.tile([128, 1], F32, tag="rcp", name=f"rcpo{s}")
                    nc.vector.reciprocal(rcp, pvo[:, s, NCLASS:NCLASS + 1])
                    nc.vector.tensor_scalar_mul(zoall[:, s, :],
                                                pvo[:, s, 0:NCLASS], rcp)
                ziall = awork.tile([128, NSUB, NCLASS], F32, tag="ziall")
                _elu_into(nc, awork, ziall, zoall, "oall")
                ls = awork.tile([128, NSUB, NCLASS], F32, tag="ls", name="ls")
                for s in range(NSUB):
                    zi = ziall[:, s, :]
                    edump = awork.tile([128, NCLASS], F32, tag="edump",
                                       name=f"ed{s}")
                    ssum = awork.tile([128, 1], F32, tag="ssum", name=f"ss{s}")
                    nc.scalar.activation(edump, zi, AF.Exp, accum_out=ssum)
                    lns = awork.tile([128, 1], F32, tag="lns", name=f"ln{s}")
                    nc.scalar.activation(lns, ssum, AF.Ln)
                    nc.vector.tensor_scalar(ls[:, s, :], zi, scalar1=lns,
                                            scalar2=None, op0=OP.subtract)
                nc.sync.dma_start(
                    out=outb[:].rearrange("(c p) f -> p c f", p=128), in_=ls)

    _split_sync_waits(nc)
    return nc


_NC_CACHE = None

_GATE_BASE = {0: 0, 1: 32, 2: 96, 3: 64}  # pytorch i,f,g,o -> partition base


def _pack_wih(w):
    """[4H, in] -> block-diag packed [2*in, 128] bf16: group0 inputs at rows
    0:in -> gate cols base+0:12; group1 at rows in:2*in -> base+12:24."""
    w = np.asarray(w, np.float32)
    nin = w.shape[1]
    out = np.zeros((2 * nin, 128), np.float32)
    for k in range(4):
        base = _GATE_BASE[k]
        blk = w[LH * k:LH * (k + 1), :].T  # [in, 12]
        out[0:nin, base:base + LH] = blk
        out[nin:2 * nin, base + LH:base + 2 * LH] = blk
    return out.astype(ml_dtypes.bfloat16)


def _pack_bias(ba, bb):
    b = np.asarray(ba, np.float32) + np.asarray(bb, np.float32)
    out = np.zeros((128, 1), np.float32)
    for k in range(4):
        base = _GATE_BASE[k]
        out[base:base + LH, 0] = b[LH * k:LH * (k + 1)]
        out[base + LH:base + 2 * LH, 0] = b[LH * k:LH * (k + 1)]
    return out


def kernel(x, adj, Wih0, Whh0, bih0, bhh0, Wih1, Whh1, bih1, bhh1,
           W_heads, a_heads, W_out, a_out):
    global _NC_CACHE
    if _NC_CACHE is None:
        _NC_CACHE = _build_program()
    nc = _NC_CACHE

    x = np.asarray(x, np.float32)
    adj = np.asarray(adj, np.int32)
    W_heads = np.asarray(W_heads, np.float32)
    a_heads = np.asarray(a_heads, np.float32)
    W_out = np.asarray(W_out, np.float32)
    a_out = np.asarray(a_out, np.float32)

    wcat = np.concatenate(
        [W_heads,
         W_heads @ a_heads[:, :NHID, :],
         W_heads @ a_heads[:, NHID:, :]],
        axis=2).astype(ml_dtypes.float8_e4m3fn)
    # f1 coef at col 16 (used for ub_o), f2 coef at col 17: pwo then carries
    # f2o at col 17 which g2stage forwards as gather column 17
    wocat = np.concatenate(
        [W_out, W_out @ a_out[:NCLASS], W_out @ a_out[NCLASS:]],
        axis=1).astype(ml_dtypes.bfloat16)

    lwts = np.zeros((128, 128), np.float32)
    lwts[0:2 * NIN] = _pack_wih(Wih0).astype(np.float32)
    lwts[32:56] = _pack_wih(Whh0).astype(np.float32)
    lwts[64:88] = _pack_wih(Wih1).astype(np.float32)
    lwts[96:120] = _pack_wih(Whh1).astype(np.float32)
    bds = np.concatenate([_pack_bias(bih0, bhh0),
                          _pack_bias(bih1, bhh1)], axis=1)
    common = {
        "lwts": lwts,
        "bds": np.ascontiguousarray(bds.astype(np.float32)),
        "wcat": np.ascontiguousarray(wcat),
        "wocat": np.ascontiguousarray(wocat),
    }
    adjT = adj.T.astype(ml_dtypes.bfloat16)  # [N(cols j), N(rows i)]
    in_maps = []
    for i in range(NCORES):
        blk = slice(R * i, R * (i + 1))
        xb = x[blk]  # [512, 8, 2]
        xpk = np.concatenate(
            [xb[0:RH].transpose(2, 1, 0), xb[RH:R].transpose(2, 1, 0)],
            axis=0)  # [4, 8, 256]
        in_maps.append({
            "xp": np.ascontiguousarray(xpk).astype(ml_dtypes.bfloat16),
            "adjTb": np.ascontiguousarray(adjT[:, blk]),
            **common,
        })

    res = run_bass_kernel_spmd(nc, in_maps, list(range(NCORES)), **_RUN_KWARGS)
    global _LAST_RESULTS
    _LAST_RESULTS = res
    return np.concatenate([res.results[i]["outb"] for i in range(NCORES)],
                          axis=0)


_RUN_KWARGS = {}
_LAST_RESULTS = None


# revision 38
# speedup vs baseline: 1.0005x; 1.0005x over previous
"""Trainium2 Bass kernel for nn_GAT_with_LSTM (2-layer LSTM -> 8-head GAT -> GAT out).

Sharding: node/row dimension split across 8 cores (512 rows each).

Key algebraic restructuring of the GAT attention (vs. direct
exp(leakyrelu(f1+f2)) evaluation): with z = f1_i + f2_j and slope a,
    leakyrelu(z) = max(z, a*z)  =>  e = exp(lrelu(z)) = max(exp(z), exp(a*z)).
Softmax rows are invariant to any per-row (i) factor, so divide by
exp(a*f1_i):
    e'_ij = max(u_i * V_j, D_j),   u = exp((1-a)*f1), V = exp(f2), D = exp(a*f2).
This removes every full-matrix transcendental: exp() runs only on the rank-1
factors. Per 128-column chunk the e-row-block is built one of three ways,
chosen to balance engines:
  - DVE:  t = (ub * V_j) max D_j      (dual-op tensor_scalar, bf16 4x mode)
  - Pool: same op at 1x
  - ACT:  r = relu(V_j * ub - D_j)    (per-partition scale/bias APs); the
          missing mask*D_j term is added back on the PE as
          maskT_chunk @ (D (*) wpv), exact since mask is 0/1:
          mask*max(uV,D) = mask*r + mask*D.
The mask multiply runs as two tensor_tensors (DVE chunks 0:4 at bf16 2x,
Pool chunks 4:8), and the PE accumulates numerator and denominator together
(wpv's last column is ones).

The mask arrives host-side pre-transposed and pre-cast to bf16 (adj[blk].T),
loaded at t=0 while the LSTM runs. The LSTM packs two 256-node groups into
the partition dim with block-diagonal host-packed weights (one sigmoid op
covers i+f+o), runs bf16 matmuls, and software-pipelines layer 1 one step
behind layer 0. LSTM h outputs accumulate in SBUF so 4 batched DMAs
publish g1in for the AllGather (the baseline used 32 serialized stores).
The LSTM feature AllGather ships fp8e4 (f1/f2 logit noise ~0.05 and Wh value
noise average out across the ~2048-wide attention sums).

Activation tables (Sigmoid/Tanh at t=0; Exp/Ln/Relu during the g1 gather)
are preloaded with dummy ops so table loads stay off the critical path.

Head prep (Wh + f2 factors) is pipelined at attention-group granularity:
heads 0/1 prep interleaves with their own attention groups right after the
gather lands; later heads prep 1-2 groups ahead inside the pair loop. The
output-layer projection (pwo) and its gather payload accumulate
incrementally per head-pair on the PE.

Softmax max-subtraction is skipped: attention logits are O(1) (0.1-scale
Xavier weights), exp cannot overflow, softmax is shift-invariant.
"""

import json

import numpy as np
import ml_dtypes

import bass_rust
import concourse.bass as bass
import concourse.tile as tile
from concourse import mybir
from concourse.bass_utils import run_bass_kernel_spmd
from concourse.masks import make_identity

F32 = mybir.dt.float32
BF16 = mybir.dt.bfloat16
FP8 = mybir.dt.float8e4
I32 = mybir.dt.int32
AF = mybir.ActivationFunctionType
OP = mybir.AluOpType

NCORES = 8
N = 4096
R = N // NCORES          # 512 rows per core
SEQ, NIN, LH = 8, 2, 12
FEAT = SEQ * LH          # 96
NHID, NHEADS, NCLASS = 64, 8, 16
ALPHA = 0.2
NJC = N // 128           # 32 j-chunks
NSUB = R // 128          # 4 row sub-blocks per core
GRP = 8                  # j-chunks per group
RH = R // 2              # 256-node half (LSTM partition packing)
NGRP = NJC // GRP        # 4 groups
GOUT = NCLASS + 2        # gathered g2 row: [Whout(16) | ones | f2o]

# chunk-q assignment within each GRP of 8 chunks. The wide mask-mul TT runs
# solo on DVE (one instruction at the bf16 2x rate); splitting it or moving
# tq builds onto DVE loses to per-instruction overhead.
Q_ACT = (5, 6, 7)        # tq via ACT relu (needs mask*D PE correction)
Q_DVE = ()               # remaining tq on Pool


def _split_sync_waits(nc, max_waits=1):
    """This walrus build rejects >1 sync wait per TPB_CTRL instruction
    ("Too many sync wait commands"). Move excess waits onto NoOps inserted
    just before; same-engine program order preserves the semantics."""
    m = json.loads(bass_rust.module_to_json_string(nc.m))
    ctr = 0
    for fn in m["functions"]:
        for bb in fn["blocks"]:
            out = []
            for inst in bb["instructions"]:
                si = inst.get("sync_info")
                ow = (si or {}).get("on_wait") or []
                if len(ow) > max_waits:
                    excess, keep = ow[:-max_waits], ow[-max_waits:]
                    for i in range(0, len(excess), max_waits):
                        ctr += 1
                        out.append({
                            "engine": inst["engine"], "ins": [], "outs": [],
                            "name": f"wsplit-{ctr}", "opcode": "NoOp",
                            "sync_info": {"on_update": [],
                                          "on_wait": excess[i:i + max_waits]},
                        })
                    si["on_wait"] = keep
                out.append(inst)
            bb["instructions"] = out
    nc.m = bass_rust.module_from_json_bytes(json.dumps(m).encode())


RQ = RH // 2  # 128-node quarter: free-dim half of a packed 256 pair


class _LstmState:
    def __init__(self, lay, p1, wih, whh, b, xin):
        self.lay, self.wih, self.whh, self.b, self.xin = lay, wih, whh, b, xin
        # c lives at partition base 32 so TensorTensor partners the f-gate
        # slice (walrus requires equal SBUF base partitions for both inputs)
        self.c01 = [p1.tile([56, RQ], F32, tag=f"c{lay}_{hf}",
                            name=f"c{lay}_{hf}")[32:56, :] for hf in (0, 1)]
        self.hprev = [None, None]


def _lstm_step(nc, st, t, hf, psg, lwork, hpool):
    """One packed LSTM step for free-half hf. Partition layout (two 256-node
    groups packed): i at rows 0:24, f at 32:56, o at 64:88, g at 96:120.
    One sigmoid covers i/f/o. The two free-halves are independent chains, so
    four chains (2 layers x 2 halves) pipeline across the engines."""
    lay = st.lay
    g = psg.tile([128, RQ], F32, tag=f"g{hf}", name=f"g{lay}_{t}_{hf}")
    nc.tensor.matmul(g, st.wih, st.xin(t, hf), start=True, stop=(t == 0))
    if t > 0:
        nc.tensor.matmul(g, st.whh, st.hprev[hf], start=False, stop=True)
    sfi = lwork.tile([88, RQ], F32, tag=f"sfi{hf}", name=f"sfi{lay}_{t}_{hf}")
    nc.scalar.activation(sfi, g[0:88, :], AF.Sigmoid, bias=st.b[0:88, :])
    tg = lwork.tile([24, RQ], F32, tag=f"tg{hf}", name=f"tg{lay}_{t}_{hf}")
    nc.scalar.activation(tg, g[96:120, :], AF.Tanh, bias=st.b[96:120, :])
    c01 = st.c01[hf]
    if t == 0:
        nc.gpsimd.tensor_mul(c01, sfi[0:24, :], tg)
    else:
        # ig at base 32 to partner c01; th at base 64 to partner the o slice.
        # c-mul on Pool, ig/c-add split keeps the DVE (the LSTM-phase
        # bottleneck) down to one op per step.
        ig = lwork.tile([56, RQ], F32, tag=f"ig{hf}",
                        name=f"ig{lay}_{t}_{hf}")[32:56, :]
        nc.gpsimd.tensor_mul(ig, sfi[0:24, :], tg)
        nc.gpsimd.tensor_mul(c01, sfi[32:56, :], c01)
        nc.vector.tensor_add(c01, c01, ig)
    th = lwork.tile([88, RQ], F32, tag=f"th{hf}",
                    name=f"th{lay}_{t}_{hf}")[64:88, :]
    nc.scalar.activation(th, c01, AF.Tanh)
    h = hpool.tile([24, RQ], BF16, tag=f"h{lay}_{hf}", name=f"h{lay}_{t}_{hf}")
    nc.vector.tensor_mul(h, sfi[64:88, :], th)
    st.hprev[hf] = h
    return h


def _attention_group(nc, awork, pv, cg, ub, vcols, dcols, negd, wpv, wpv_d,
                     maskT, pfx):
    """One GRP-chunk group of the masked-softmax attention accumulation."""
    tq = awork.tile([128, GRP, R], BF16, tag="tq", name=f"tq_{pfx}_{cg}")
    for q in range(GRP):
        c = cg * GRP + q
        if q in Q_ACT:
            nc.scalar.activation(tq[:, q, :], ub, AF.Relu,
                                 scale=vcols[:, c, :],
                                 bias=negd[:, c, :])
        else:
            eng = nc.vector if q in Q_DVE else nc.gpsimd
            eng.tensor_scalar(tq[:, q, :], ub, scalar1=vcols[:, c, :],
                              scalar2=dcols[:, c, :],
                              op0=OP.mult, op1=OP.max)
    e3 = awork.tile([128, GRP, R], BF16, tag="e3", name=f"e3_{pfx}_{cg}")
    nc.vector.tensor_mul(e3, tq, maskT[:, cg * GRP:(cg + 1) * GRP, :])
    last = cg == NGRP - 1
    for q in range(GRP):
        c = cg * GRP + q
        for s in range(NSUB):
            nc.tensor.matmul(pv[:, s, :], e3[:, q, 128 * s:128 * (s + 1)],
                             wpv[:, c, :], start=(c == 0),
                             stop=(last and q == GRP - 1))
        if q in Q_ACT:
            # mask*D_j correction for the relu form
            for s in range(NSUB):
                nc.tensor.matmul(pv[:, s, :],
                                 maskT[:, c, 128 * s:128 * (s + 1)],
                                 wpv_d[:, cg * len(Q_ACT) + Q_ACT.index(q), :],
                                 start=False, stop=False)


def _elu_into(nc, awork, dst, z, pfx):
    """dst = elu(z) = min(exp(z),1)-1 + max(z,0), elementwise."""
    ez = awork.tile(list(z.shape), F32, tag="elu_ez", name=f"ez_{pfx}")
    nc.scalar.activation(ez, z, AF.Exp)
    nc.gpsimd.tensor_scalar(ez, ez, scalar1=1.0, scalar2=-1.0,
                            op0=OP.min, op1=OP.add)
    zr = awork.tile(list(z.shape), F32, tag="elu_zr", name=f"zr_{pfx}")
    nc.scalar.activation(zr, z, AF.Relu)
    nc.gpsimd.tensor_add(dst, ez, zr)


def _ubcast(nc, psf1, awork, ubpool, ones1, coefT, feats, nk, scale, pfx):
    """u = exp(scale * (coefT.T @ feats)) broadcast over partitions."""
    pf1 = psf1.tile([1, R], F32, tag="f1r", name=f"pf1_{pfx}")
    if nk == 1:
        nc.tensor.matmul(pf1, coefT, feats, start=True, stop=True)
    else:
        for fc in range(nk):
            nc.tensor.matmul(pf1, coefT[:, fc, :], feats[:, fc, :],
                             start=(fc == 0), stop=(fc == nk - 1))
    f1row = awork.tile([1, R], BF16, tag="f1row", name=f"f1row_{pfx}")
    nc.scalar.copy(f1row, pf1)
    pf1b = psf1.tile([128, R], F32, tag="f1r", name=f"pf1b_{pfx}")
    nc.tensor.matmul(pf1b, ones1, f1row, start=True, stop=True)
    ub = ubpool.tile([128, R], BF16, tag=f"ub_{pfx}", name=f"ub_{pfx}")
    nc.scalar.activation(ub, pf1b, AF.Exp, scale=scale)
    return ub


def _build_program():
    nc = bass.Bass()

    xp = nc.dram_tensor("xp", [2 * NIN, SEQ, RH], BF16, kind="ExternalInput")
    adjTb = nc.dram_tensor("adjTb", [N, R], BF16, kind="ExternalInput")
    lwts = nc.dram_tensor("lwts", [128, 128], F32, kind="ExternalInput")
    bds = nc.dram_tensor("bds", [128, 2], F32, kind="ExternalInput")
    wcat = nc.dram_tensor("wcat", [NHEADS, FEAT, NHID + 2], FP8,
                          kind="ExternalInput")
    wocat = nc.dram_tensor("wocat", [NHEADS * NHID, NCLASS + 2], BF16,
                           kind="ExternalInput")
    outb = nc.dram_tensor("outb", [R, NCLASS], F32, kind="ExternalOutput")

    with tile.TileContext(nc) as tc:
        with tc.tile_pool(name="cst", bufs=1) as cst, \
             tc.tile_pool(name="dram", bufs=1, space="DRAM") as dram:

            ident = cst.tile([128, 128], BF16)
            make_identity(nc, ident)
            ones1 = cst.tile([1, 128], BF16)
            nc.vector.memset(ones1, 1.0)
            maskT = cst.tile([128, NJC, R], BF16)
            hT_own = cst.tile([FEAT, R], FP8)
            # gathered features split in two tiles so group-0 prep only
            # waits on the first (smaller) load
            hT_a = cst.tile([FEAT, 2, R], FP8)
            hT_b = cst.tile([FEAT, NCORES - 2, R], FP8)
            wcsb = cst.tile([FEAT, NHEADS, NHID + 2], FP8)
            wocsb = cst.tile([128, NSUB, NCLASS + 2], BF16)
            dumA = cst.tile([1, 2], F32)
            dumB = cst.tile([1, 2], F32)

            g1in = dram.tile([FEAT, R], FP8)
            g1out = dram.tile([NCORES * FEAT, R], FP8, addr_space="Shared")
            g2in = dram.tile([R, GOUT], BF16)
            g2out = dram.tile([N, GOUT], BF16, addr_space="Shared")

            # ======== Phase 1: LSTM (own nodes, 2 groups packed) ===========
            with tc.tile_pool(name="p1", bufs=1) as p1, \
                 tc.tile_pool(name="psg", bufs=4, space="PSUM") as psg, \
                 tc.tile_pool(name="hpool0", bufs=SEQ) as hpool0, \
                 tc.tile_pool(name="hpool1", bufs=3) as hpool1, \
                 tc.tile_pool(name="lwork", bufs=6) as lwork:

                xp_sb = p1.tile([2 * NIN, SEQ, RH], BF16)
                nc.sync.dma_start(out=xp_sb, in_=xp[:])
                lw = p1.tile([128, 128], F32)
                nc.sync.dma_start(out=lw, in_=lwts[:])
                bt = p1.tile([128, 2], F32)
                nc.sync.dma_start(out=bt, in_=bds[:])
                # head weights + output weights: single DMAs, early
                nc.sync.dma_start(out=wcsb,
                                  in_=wcat[:].rearrange("h f c -> f h c"))
                nc.sync.dma_start(
                    out=wocsb, in_=wocat.rearrange("(c p) f -> p c f", p=128))
                # mask loads at t=0: SP is otherwise idle during the LSTM
                adjTr = adjTb[:].rearrange("(c p) r -> p c r", p=128)
                for mg in range(4):
                    nc.sync.dma_start(out=maskT[:, 8 * mg:8 * (mg + 1), :],
                                      in_=adjTr[:, 8 * mg:8 * (mg + 1), :])

                # preload Sigmoid/Tanh activation tables during input DMAs
                nc.vector.memset(dumA, 0.0)
                nc.scalar.activation(dumB, dumA, AF.Sigmoid)
                nc.scalar.activation(dumB, dumA, AF.Tanh)

                w0 = p1.tile([2 * NIN, 128], BF16)
                w0h = p1.tile([24, 128], BF16)
                w1 = p1.tile([24, 128], BF16)
                w1h = p1.tile([24, 128], BF16)
                nc.vector.tensor_copy(w0, lw[0:2 * NIN, :])
                nc.vector.tensor_copy(w0h, lw[32:56, :])
                nc.vector.tensor_copy(w1, lw[64:88, :])
                nc.vector.tensor_copy(w1h, lw[96:120, :])
                b0 = bt[:, 0:1]
                b1 = bt[:, 1:2]

                st0 = _LstmState(0, p1, w0, w0h, b0,
                                 lambda t, hf: xp_sb[:, t, RQ * hf:RQ * (hf + 1)])
                h0s = [[], []]
                st1 = _LstmState(1, p1, w1, w1h, b1,
                                 lambda t, hf: h0s[hf][t])

                # software-pipeline: layer 1 runs one step behind layer 0;
                # the two free-halves are independent chains, so four chains
                # interleave on every engine queue. Layer-1 h accumulates in
                # SBUF (free-dim placement keeps partition bases legal) so
                # only 4 batched DMAs publish g1in at the end.
                hacc = [p1.tile([24, SEQ, RQ], FP8, name=f"hacc{hf}")
                        for hf in (0, 1)]
                # g1in[12t+l, 128*(2g+hf)+c] = hacc[hf][12g+l, t, c]
                g1v = g1in[:].rearrange("(t l) (b c) -> l t b c", l=LH, b=4)
                for slot in range(SEQ + 1):
                    for hf in (0, 1):
                        if slot < SEQ:
                            h0s[hf].append(_lstm_step(nc, st0, slot, hf, psg,
                                                      lwork, hpool0))
                    for hf in (0, 1):
                        if slot >= 1:
                            t = slot - 1
                            h1 = _lstm_step(nc, st1, t, hf, psg, lwork,
                                            hpool1)
                            nc.gpsimd.tensor_copy(hacc[hf][:, t, :], h1)
                    if slot == SEQ - 2:
                        # steps 0..5 are final: publish them while steps 6-7
                        # compute, leaving only tiny stores at the end
                        for g in (0, 1):
                            for hf in (0, 1):
                                nc.sync.dma_start(
                                    out=g1v[:, 0:SEQ - 2, 2 * g + hf, :],
                                    in_=hacc[hf][LH * g:LH * (g + 1),
                                                 0:SEQ - 2, :])
                # final 2 steps: 2 stores on SP + 2 on Act so they drain in
                # parallel right as the last h lands
                for gi, (g, hf) in enumerate(((0, 0), (0, 1), (1, 0), (1, 1))):
                    eng = nc.sync if gi < 2 else nc.scalar
                    eng.dma_start(
                        out=g1v[:, SEQ - 2:SEQ, 2 * g + hf, :],
                        in_=hacc[hf][LH * g:LH * (g + 1), SEQ - 2:SEQ, :])
                nc.gpsimd.collective_compute(
                    "AllGather", OP.bypass,
                    replica_groups=[list(range(NCORES))],
                    ins=[g1in[:].opt()],
                    outs=[g1out[:].opt()])
                nc.sync.dma_start(out=hT_own, in_=g1in)

            # ======== Phase 2: 8 GAT heads + output GAT layer ===============
            with tc.tile_pool(name="att", bufs=1) as att, \
                 tc.tile_pool(name="pstr", bufs=1, space="PSUM") as pstr, \
                 tc.tile_pool(name="pswh", bufs=3, space="PSUM") as pswh, \
                 tc.tile_pool(name="psf1", bufs=1, space="PSUM") as psf1, \
                 tc.tile_pool(name="psout", bufs=1, space="PSUM") as psout, \
                 tc.tile_pool(name="pspv", bufs=2, space="PSUM") as pspv, \
                 tc.tile_pool(name="hw", bufs=4) as hw, \
                 tc.tile_pool(name="awork", bufs=4) as awork:

                hcat = att.tile([128, NSUB, NHEADS * NHID], BF16)
                hcatT = att.tile([128, NSUB, R], BF16)

                # preload Exp/Ln/Relu tables + u for all 8 heads (overlaps
                # the g1 AllGather latency: needs only hT_own)
                nc.scalar.activation(dumB, dumA, AF.Exp)
                nc.scalar.activation(dumB, dumA, AF.Ln)
                nc.scalar.activation(dumB, dumA, AF.Relu)
                ubs = []
                for h in range(NHEADS):
                    ubs.append(_ubcast(nc, psf1, awork, att, ones1,
                                       wcsb[:, h, NHID:NHID + 1], hT_own, 1,
                                       1.0 - ALPHA, f"h{h}"))

                # gathered features: 2 DMAs so head-0 prep starts early
                g1r = g1out[:].rearrange("(b f) r -> f b r", f=FEAT)
                nc.sync.dma_start(out=hT_a, in_=g1r[:, 0:2, :])
                nc.sync.dma_start(out=hT_b, in_=g1r[:, 2:NCORES, :])

                def _head_prep_start(h):
                    whpv = hw.tile([128, NJC, NHID + 1], BF16, tag="whpv",
                                   name=f"whpv{h}")
                    nc.vector.memset(whpv[:, :, NHID:NHID + 1], 1.0)
                    f2cols = hw.tile([128, NJC, 1], F32, tag="f2cols",
                                     name=f"f2cols{h}")
                    vcols = hw.tile([128, NJC, 1], F32, tag="vcols",
                                    name=f"vcols{h}")
                    dcols = hw.tile([128, NJC, 1], F32, tag="dcols",
                                    name=f"dcols{h}")
                    negd = hw.tile([128, NJC, 1], F32, tag="negd",
                                   name=f"negd{h}")
                    wpvd = hw.tile([128, NGRP * len(Q_ACT), NHID + 1], BF16,
                                   tag="wpvd", name=f"wpvd{h}")
                    return [whpv, f2cols, vcols, dcols, negd, wpvd, None,
                            None]

                def _head_prep_piece(h, st, gq):
                    # Wh (+f2) for 4 chunks; small pieces keep the in-order
                    # ACT queue smooth so attention relu-P1s are not stalled
                    # behind a prep burst
                    whpv, f2cols = st[0], st[1]
                    pw4 = pswh.tile([128, 4, NHID + 2], F32, tag="wh",
                                    name=f"pw{h}_{gq}")
                    for k in range(4):
                        c = 4 * gq + k
                        blk, co = c // 4, 128 * (c % 4)
                        src = (hT_a[:, blk, co:co + 128] if blk < 2
                               else hT_b[:, blk - 2, co:co + 128])
                        nc.tensor.matmul(pw4[:, k, :], src,
                                         wcsb[:, h, :], start=True, stop=True)
                    nc.scalar.copy(whpv[:, 4 * gq:4 * (gq + 1), 0:NHID],
                                   pw4[:, :, 0:NHID])
                    nc.scalar.copy(f2cols[:, 4 * gq:4 * (gq + 1), :],
                                   pw4[:, :, NHID + 1:NHID + 2])

                def _factors_group(h, st, cg):
                    # V/D/negD for chunk-group cg (pieces 2cg, 2cg+1); the
                    # wpvd scalings are emitted just-in-time at the consuming
                    # attention group so they never head-of-line block DVE
                    whpv, f2cols, vcols, dcols, negd, wpvd = st[:6]
                    cs = slice(GRP * cg, GRP * (cg + 1))
                    nc.scalar.activation(vcols[:, cs, :], f2cols[:, cs, :],
                                         AF.Exp)
                    nc.scalar.activation(dcols[:, cs, :], f2cols[:, cs, :],
                                         AF.Exp, scale=ALPHA)
                    nc.gpsimd.tensor_scalar(negd[:, cs, :], dcols[:, cs, :],
                                            scalar1=-1.0, scalar2=None,
                                            op0=OP.mult)

                def _wpvd_jit(st, cg):
                    whpv, _, _, dcols, _, wpvd = st[:6]
                    for qi, q in enumerate(Q_ACT):
                        c = cg * GRP + q
                        nc.vector.tensor_scalar_mul(
                            wpvd[:, cg * len(Q_ACT) + qi, :],
                            whpv[:, c, :], dcols[:, c, :])

                def _prep_task(h, st, j):
                    _head_prep_piece(h, st, 2 * j)
                    _head_prep_piece(h, st, 2 * j + 1)
                    _factors_group(h, st, j)

                def _prep_pe(h, st, j):
                    # PE half of a prep task: Wh matmuls for pieces 2j, 2j+1
                    # (slots into the PE idle window under the mask-mul TT)
                    for gq in (2 * j, 2 * j + 1):
                        pw4 = pswh.tile([128, 4, NHID + 2], F32, tag="wh",
                                        name=f"pw{h}_{gq}")
                        for k in range(4):
                            c = 4 * gq + k
                            blk, co = c // 4, 128 * (c % 4)
                            src = (hT_a[:, blk, co:co + 128] if blk < 2
                                   else hT_b[:, blk - 2, co:co + 128])
                            nc.tensor.matmul(pw4[:, k, :], src,
                                             wcsb[:, h, :], start=True,
                                             stop=True)
                        st[6 + gq % 2] = pw4

                def _prep_act(h, st, j):
                    # ACT half: PSUM->SBUF copies + factor exps
                    whpv, f2cols = st[0], st[1]
                    for gq in (2 * j, 2 * j + 1):
                        pw4 = st[6 + gq % 2]
                        nc.scalar.copy(whpv[:, 4 * gq:4 * (gq + 1), 0:NHID],
                                       pw4[:, :, 0:NHID])
                        nc.scalar.copy(f2cols[:, 4 * gq:4 * (gq + 1), :],
                                       pw4[:, :, NHID + 1:NHID + 2])
                    _factors_group(h, st, j)

                def _head_post(h, pv):
                    zall = awork.tile([128, NSUB, NHID], F32, tag="zall",
                                      name=f"zall{h}")
                    for s in range(NSUB):
                        rcp = awork.tile([128, 1], F32, tag="rcp",
                                         name=f"rcp{h}_{s}")
                        nc.vector.reciprocal(rcp, pv[:, s, NHID:NHID + 1])
                        nc.vector.tensor_scalar_mul(zall[:, s, :],
                                                    pv[:, s, 0:NHID], rcp)
                    _elu_into(nc, awork, hcat[:, :, NHID * h:NHID * (h + 1)],
                              zall, f"h{h}")

                # output-layer projection: per-pair PSUM groups (a PE
                # accumulation group must not stay open across interleaved
                # transposes), summed into an SBUF accumulator
                pwacc = att.tile([128, NSUB, NCLASS + 2], F32)
                pwos = {}

                def _transpose_piece(hp, s):
                    ptr = pstr.tile([128, 128], BF16, tag="tr",
                                    name=f"trp{hp}_{s}")
                    nc.tensor.transpose(
                        ptr, hcat[:, s, 128 * hp:128 * (hp + 1)], ident)
                    nc.scalar.copy(hcatT[:, hp, 128 * s:128 * (s + 1)], ptr)
                    if hp not in pwos:
                        pwos[hp] = psout.tile([128, NSUB, NCLASS + 2], F32,
                                              tag="pwo", name=f"pwo{hp}")
                    nc.tensor.matmul(pwos[hp][:, s, :],
                                     hcatT[:, hp, 128 * s:128 * (s + 1)],
                                     wocsb[:, hp, :], start=True, stop=True)

                def _pwo_accum(hp):
                    if hp == 0:
                        nc.vector.tensor_copy(pwacc, pwos[hp])
                    else:
                        nc.vector.tensor_add(pwacc, pwacc, pwos[hp])
                    del pwos[hp]

                def _prep_for(h, j, fn):
                    if h >= NHEADS or j > 3:
                        return
                    if h not in sts:
                        sts[h] = _head_prep_start(h)
                    fn(h, sts[h], j)

                sts = {0: _head_prep_start(0), 1: _head_prep_start(1)}
                _prep_task(0, sts[0], 0)
                _prep_task(1, sts[1], 0)
                _prep_task(0, sts[0], 1)
                _prep_task(1, sts[1], 1)

                # prep emission: a full pair of lookahead. PE halves of
                # pair hp+1's tasks go BEFORE the slot's attention group
                # (the PE is idle under the mask-mul TT); ACT halves go
                # after the first head's group so they never delay the
                # current slot's relu builds. Pair 0's own heads are
                # just-in-time whole tasks.
                for hp in range(NHEADS // 2):
                    ha, hb = 2 * hp, 2 * hp + 1
                    pa = sts[ha]
                    pb = sts[hb]
                    pva = pspv.tile([128, NSUB, NHID + 1], F32, tag="pv",
                                    name=f"pv_h{ha}")
                    pvb = pspv.tile([128, NSUB, NHID + 1], F32, tag="pv",
                                    name=f"pv_h{hb}")
                    for cg in range(NGRP):
                        if hp == 0 and 1 <= cg < 3:
                            _prep_for(ha, cg + 1, _prep_task)
                        _prep_for(ha + 2, cg, _prep_pe)
                        _wpvd_jit(pa, cg)
                        _attention_group(nc, awork, pva, cg, ubs[ha], pa[2],
                                         pa[3], pa[4], pa[0], pa[5], maskT,
                                         f"h{ha}")
                        _prep_for(ha + 2, cg, _prep_act)
                        if hp == 0 and 1 <= cg < 3:
                            _prep_for(hb, cg + 1, _prep_task)
                        _prep_for(hb + 2, cg, _prep_pe)
                        _wpvd_jit(pb, cg)
                        _attention_group(nc, awork, pvb, cg, ubs[hb], pb[2],
                                         pb[3], pb[4], pb[0], pb[5], maskT,
                                         f"h{hb}")
                        _prep_for(hb + 2, cg, _prep_act)
                        if hp >= 1:
                            # deferred: previous pair's hcatT transpose +
                            # output-projection piece (one sub-block per
                            # slot). Deferring keeps these off the PE queue
                            # head at the pair boundary, where they would
                            # stall the next pair behind the elu chain.
                            _transpose_piece(hp - 1, cg)
                    del sts[ha], sts[hb]
                    _head_post(ha, pva)
                    _head_post(hb, pvb)
                    if hp >= 1:
                        _pwo_accum(hp - 1)

                for s in range(NSUB):
                    _transpose_piece(NHEADS // 2 - 1, s)
                _pwo_accum(NHEADS // 2 - 1)

                # ---- publish output-layer gather payload ----
                # g2 row: [Whout(0:16) | ones(16) | f2o(17)]
                g2stage = awork.tile([128, NSUB, GOUT], BF16, tag="g2stage")
                nc.vector.memset(g2stage[:, :, NCLASS:NCLASS + 1], 1.0)
                nc.scalar.copy(g2stage[:, :, 0:NCLASS], pwacc[:, :, 0:NCLASS])
                nc.scalar.copy(g2stage[:, :, NCLASS + 1:NCLASS + 2],
                               pwacc[:, :, NCLASS + 1:NCLASS + 2])
                nc.sync.dma_start(
                    out=g2in[:].rearrange("(c p) f -> p c f", p=128),
                    in_=g2stage)

                ub_o = _ubcast(nc, psf1, awork, att, ones1,
                               wocsb[:, :, NCLASS:NCLASS + 1], hcatT, NSUB,
                               1.0 - ALPHA, "o")

                nc.gpsimd.collective_compute(
                    "AllGather", OP.bypass,
                    replica_groups=[list(range(NCORES))],
                    ins=[g2in[:].opt()], outs=[g2out[:].opt()])

                # ---- output attention (pipelined per group) ----
                g2full = hw.tile([128, NJC, GOUT], BF16, tag="g2full",
                                 name="g2full")
                vocols = hw.tile([128, NJC, 1], F32, tag="vcols",
                                 name="vocols")
                docols = hw.tile([128, NJC, 1], F32, tag="dcols",
                                 name="docols")
                negdo = hw.tile([128, NJC, 1], F32, tag="negd", name="negdo")
                wpvdo = hw.tile([128, NGRP * len(Q_ACT), NCLASS + 1],
                                BF16, tag="wpvd", name="wpvdo")
                pvo = pspv.tile([128, NSUB, NCLASS + 1], F32, tag="pv",
                                name="pv_o")
                g2r = g2out[:].rearrange("(c p) f -> p c f", p=128)
                for cg in range(NGRP):
                    cs = slice(GRP * cg, GRP * (cg + 1))
                    nc.sync.dma_start(out=g2full[:, cs, :], in_=g2r[:, cs, :])
                    nc.scalar.activation(vocols[:, cs, :],
                                         g2full[:, cs, NCLASS + 1:NCLASS + 2],
                                         AF.Exp)
                    nc.scalar.activation(docols[:, cs, :],
                                         g2full[:, cs, NCLASS + 1:NCLASS + 2],
                                         AF.Exp, scale=ALPHA)
                    nc.gpsimd.tensor_scalar(negdo[:, cs, :], docols[:, cs, :],
                                            scalar1=-1.0, scalar2=None,
                                            op0=OP.mult)
                    for qi, q in enumerate(Q_ACT):
                        c = cg * GRP + q
                        nc.vector.tensor_scalar_mul(
                            wpvdo[:, cg * len(Q_ACT) + qi, :],
                            g2full[:, c, 0:NCLASS + 1], docols[:, c, :])
                    _attention_group(nc, awork, pvo, cg, ub_o, vocols,
                                     docols, negdo,
                                     g2full[:, :, 0:NCLASS + 1], wpvdo,
                                     maskT, "o")
                zoall = awork.tile([128, NSUB, NCLASS], F32, tag="zoall")
                for s in range(NSUB):
                    rcp = awork.tile([128, 1], F32, tag="rcp", name=f"rcpo{s}")
                    nc.vector.reciprocal(rcp, pvo[:, s, NCLASS:NCLASS + 1])
                    nc.vector.tensor_scalar_mul(zoall[:, s, :],
                                                pvo[:, s, 0:NCLASS], rcp)
                ziall = awork.tile([128, NSUB, NCLASS], F32, tag="ziall")
                _elu_into(nc, awork, ziall, zoall, "oall")
                ls = awork.tile([128, NSUB, NCLASS], F32, tag="ls", name="ls")
                for s in range(NSUB):
                    zi = ziall[:, s, :]
                    edump = awork.tile([128, NCLASS], F32, tag="edump",
                                       name=f"ed{s}")
                    ssum = awork.tile([128, 1], F32, tag="ssum", name=f"ss{s}")
                    nc.scalar.activation(edump, zi, AF.Exp, accum_out=ssum)
                    lns = awork.tile([128, 1], F32, tag="lns", name=f"ln{s}")
                    nc.scalar.activation(lns, ssum, AF.Ln)
                    nc.vector.tensor_scalar(ls[:, s, :], zi, scalar1=lns,
                                            scalar2=None, op0=OP.subtract)
                nc.sync.dma_start(
                    out=outb[:].rearrange("(c p) f -> p c f", p=128), in_=ls)

    _split_sync_waits(nc)
    return nc


_NC_CACHE = None

_GATE_BASE = {0: 0, 1: 32, 2: 96, 3: 64}  # pytorch i,f,g,o -> partition base


def _pack_wih(w):
    """[4H, in] -> block-diag packed [2*in, 128] bf16: group0 inputs at rows
    0:in -> gate cols base+0:12; group1 at rows in:2*in -> base+12:24."""
    w = np.asarray(w, np.float32)
    nin = w.shape[1]
    out = np.zeros((2 * nin, 128), np.float32)
    for k in range(4):
        base = _GATE_BASE[k]
        blk = w[LH * k:LH * (k + 1), :].T  # [in, 12]
        out[0:nin, base:base + LH] = blk
        out[nin:2 * nin, base + LH:base + 2 * LH] = blk
    return out.astype(ml_dtypes.bfloat16)


def _pack_bias(ba, bb):
    b = np.asarray(ba, np.float32) + np.asarray(bb, np.float32)
    out = np.zeros((128, 1), np.float32)
    for k in range(4):
        base = _GATE_BASE[k]
        out[base:base + LH, 0] = b[LH * k:LH * (k + 1)]
        out[base + LH:base + 2 * LH, 0] = b[LH * k:LH * (k + 1)]
    return out


def kernel(x, adj, Wih0, Whh0, bih0, bhh0, Wih1, Whh1, bih1, bhh1,
           W_heads, a_heads, W_out, a_out):
    global _NC_CACHE
    if _NC_CACHE is None:
        _NC_CACHE = _build_program()
    nc = _NC_CACHE

    x = np.asarray(x, np.float32)
    adj = np.asarray(adj, np.int32)
    W_heads = np.asarray(W_heads, np.float32)
    a_heads = np.asarray(a_heads, np.float32)
    W_out = np.asarray(W_out, np.float32)
    a_out = np.asarray(a_out, np.float32)

    wcat = np.concatenate(
        [W_heads,
         W_heads @ a_heads[:, :NHID, :],
         W_heads @ a_heads[:, NHID:, :]],
        axis=2).astype(ml_dtypes.float8_e4m3fn)
    # f1 coef at col 16 (used for ub_o), f2 coef at col 17: pwo then carries
    # f2o at col 17 which g2stage forwards as gather column 17
    wocat = np.concatenate(
        [W_out, W_out @ a_out[:NCLASS], W_out @ a_out[NCLASS:]],
        axis=1).astype(ml_dtypes.bfloat16)

    lwts = np.zeros((128, 128), np.float32)
    lwts[0:2 * NIN] = _pack_wih(Wih0).astype(np.float32)
    lwts[32:56] = _pack_wih(Whh0).astype(np.float32)
    lwts[64:88] = _pack_wih(Wih1).astype(np.float32)
    lwts[96:120] = _pack_wih(Whh1).astype(np.float32)
    bds = np.concatenate([_pack_bias(bih0, bhh0),
                          _pack_bias(bih1, bhh1)], axis=1)
    common = {
        "lwts": lwts,
        "bds": np.ascontiguousarray(bds.astype(np.float32)),
        "wcat": np.ascontiguousarray(wcat),
        "wocat": np.ascontiguousarray(wocat),
    }
    adjT = adj.T.astype(ml_dtypes.bfloat16)  # [N(cols j), N(rows i)]
    in_maps = []
    for i in range(NCORES):
        blk = slice(R * i, R * (i + 1))
        xb = x[blk]  # [512, 8, 2]
        xpk = np.concatenate(
            [xb[0:RH].transpose(2, 1, 0), xb[RH:R].transpose(2, 1, 0)],
            axis=0)  # [4, 8, 256]
        in_maps.append({
            "xp": np.ascontiguousarray(xpk).astype(ml_dtypes.bfloat16),
            "adjTb": np.ascontiguousarray(adjT[:, blk]),
            **common,
        })

    res = run_bass_kernel_spmd(nc, in_maps, list(range(NCORES)), **_RUN_KWARGS)
    global _LAST_RESULTS
    _LAST_RESULTS = res
    return np.concatenate([res.results[i]["outb"] for i in range(NCORES)],
                          axis=0)


_RUN_KWARGS = {}
_LAST_RESULTS = None


# revision 39
# speedup vs baseline: 1.0048x; 1.0043x over previous
"""Trainium2 Bass kernel for nn_GAT_with_LSTM (2-layer LSTM -> 8-head GAT -> GAT out).

Sharding: node/row dimension split across 8 cores (512 rows each).

Key algebraic restructuring of the GAT attention (vs. direct
exp(leakyrelu(f1+f2)) evaluation): with z = f1_i + f2_j and slope a,
    leakyrelu(z) = max(z, a*z)  =>  e = exp(lrelu(z)) = max(exp(z), exp(a*z)).
Softmax rows are invariant to any per-row (i) factor, so divide by
exp(a*f1_i):
    e'_ij = max(u_i * V_j, D_j),   u = exp((1-a)*f1), V = exp(f2), D = exp(a*f2).
This removes every full-matrix transcendental: exp() runs only on the rank-1
factors. Per 128-column chunk the e-row-block is built one of three ways,
chosen to balance engines:
  - DVE:  t = (ub * V_j) max D_j      (dual-op tensor_scalar, bf16 4x mode)
  - Pool: same op at 1x
  - ACT:  r = relu(V_j * ub - D_j)    (per-partition scale/bias APs); the
          missing mask*D_j term is added back on the PE as
          maskT_chunk @ (D (*) wpv), exact since mask is 0/1:
          mask*max(uV,D) = mask*r + mask*D.
The mask multiply runs as two tensor_tensors (DVE chunks 0:4 at bf16 2x,
Pool chunks 4:8), and the PE accumulates numerator and denominator together
(wpv's last column is ones).

The mask arrives host-side pre-transposed and pre-cast to bf16 (adj[blk].T),
loaded at t=0 while the LSTM runs. The LSTM packs two 256-node groups into
the partition dim with block-diagonal host-packed weights (one sigmoid op
covers i+f+o), runs bf16 matmuls, and software-pipelines layer 1 one step
behind layer 0. LSTM h outputs accumulate in SBUF so 4 batched DMAs
publish g1in for the AllGather (the baseline used 32 serialized stores).
The LSTM feature AllGather ships fp8e4 (f1/f2 logit noise ~0.05 and Wh value
noise average out across the ~2048-wide attention sums).

Activation tables (Sigmoid/Tanh at t=0; Exp/Ln/Relu during the g1 gather)
are preloaded with dummy ops so table loads stay off the critical path.

Head prep (Wh + f2 factors) is pipelined at attention-group granularity:
heads 0/1 prep interleaves with their own attention groups right after the
gather lands; later heads prep 1-2 groups ahead inside the pair loop. The
output-layer projection (pwo) and its gather payload accumulate
incrementally per head-pair on the PE.

Softmax max-subtraction is skipped: attention logits are O(1) (0.1-scale
Xavier weights), exp cannot overflow, softmax is shift-invariant.
"""

import json

import numpy as np
import ml_dtypes

import bass_rust
import concourse.bass as bass
import concourse.tile as tile
from concourse import mybir
from concourse.bass_utils import run_bass_kernel_spmd
from concourse.masks import make_identity

F32 = mybir.dt.float32
BF16 = mybir.dt.bfloat16
FP8 = mybir.dt.float8e4
I32 = mybir.dt.int32
AF = mybir.ActivationFunctionType
OP = mybir.AluOpType

NCORES = 8
N = 4096
R = N // NCORES          # 512 rows per core
SEQ, NIN, LH = 8, 2, 12
FEAT = SEQ * LH          # 96
NHID, NHEADS, NCLASS = 64, 8, 16
ALPHA = 0.2
NJC = N // 128           # 32 j-chunks
NSUB = R // 128          # 4 row sub-blocks per core
GRP = 8                  # j-chunks per group
RH = R // 2              # 256-node half (LSTM partition packing)
NGRP = NJC // GRP        # 4 groups
GOUT = NCLASS + 2        # gathered g2 row: [Whout(16) | ones | f2o]

# chunk-q assignment within each GRP of 8 chunks. The wide mask-mul TT runs
# solo on DVE (one instruction at the bf16 2x rate); splitting it or moving
# tq builds onto DVE loses to per-instruction overhead.
Q_ACT = (5, 6, 7)        # tq via ACT relu (needs mask*D PE correction)
Q_DVE = ()               # remaining tq on Pool


def _split_sync_waits(nc, max_waits=1):
    """This walrus build rejects >1 sync wait per TPB_CTRL instruction
    ("Too many sync wait commands"). Move excess waits onto NoOps inserted
    just before; same-engine program order preserves the semantics."""
    m = json.loads(bass_rust.module_to_json_string(nc.m))
    ctr = 0
    for fn in m["functions"]:
        for bb in fn["blocks"]:
            out = []
            for inst in bb["instructions"]:
                si = inst.get("sync_info")
                ow = (si or {}).get("on_wait") or []
                if len(ow) > max_waits:
                    excess, keep = ow[:-max_waits], ow[-max_waits:]
                    for i in range(0, len(excess), max_waits):
                        ctr += 1
                        out.append({
                            "engine": inst["engine"], "ins": [], "outs": [],
                            "name": f"wsplit-{ctr}", "opcode": "NoOp",
                            "sync_info": {"on_update": [],
                                          "on_wait": excess[i:i + max_waits]},
                        })
                    si["on_wait"] = keep
                out.append(inst)
            bb["instructions"] = out
    nc.m = bass_rust.module_from_json_bytes(json.dumps(m).encode())


RQ = RH // 2  # 128-node quarter: free-dim half of a packed 256 pair


class _LstmState:
    def __init__(self, lay, p1, wih, whh, b, xin):
        self.lay, self.wih, self.whh, self.b, self.xin = lay, wih, whh, b, xin
        # c lives at partition base 32 so TensorTensor partners the f-gate
        # slice (walrus requires equal SBUF base partitions for both inputs)
        self.c01 = [p1.tile([56, RQ], F32, tag=f"c{lay}_{hf}",
                            name=f"c{lay}_{hf}")[32:56, :] for hf in (0, 1)]
        self.hprev = [None, None]


def _lstm_step(nc, st, t, hf, psg, lwork, hpool):
    """One packed LSTM step for free-half hf. Partition layout (two 256-node
    groups packed): i at rows 0:24, f at 32:56, o at 64:88, g at 96:120.
    One sigmoid covers i/f/o. The two free-halves are independent chains, so
    four chains (2 layers x 2 halves) pipeline across the engines."""
    lay = st.lay
    g = psg.tile([128, RQ], F32, tag=f"g{hf}", name=f"g{lay}_{t}_{hf}")
    nc.tensor.matmul(g, st.wih, st.xin(t, hf), start=True, stop=(t == 0))
    if t > 0:
        nc.tensor.matmul(g, st.whh, st.hprev[hf], start=False, stop=True)
    sfi = lwork.tile([88, RQ], F32, tag=f"sfi{hf}", name=f"sfi{lay}_{t}_{hf}")
    nc.scalar.activation(sfi, g[0:88, :], AF.Sigmoid, bias=st.b[0:88, :])
    tg = lwork.tile([24, RQ], F32, tag=f"tg{hf}", name=f"tg{lay}_{t}_{hf}")
    nc.scalar.activation(tg, g[96:120, :], AF.Tanh, bias=st.b[96:120, :])
    c01 = st.c01[hf]
    if t == 0:
        nc.gpsimd.tensor_mul(c01, sfi[0:24, :], tg)
    else:
        # ig at base 32 to partner c01; th at base 64 to partner the o slice.
        # c-mul on Pool, ig/c-add split keeps the DVE (the LSTM-phase
        # bottleneck) down to one op per step.
        ig = lwork.tile([56, RQ], F32, tag=f"ig{hf}",
                        name=f"ig{lay}_{t}_{hf}")[32:56, :]
        nc.gpsimd.tensor_mul(ig, sfi[0:24, :], tg)
        nc.gpsimd.tensor_mul(c01, sfi[32:56, :], c01)
        nc.vector.tensor_add(c01, c01, ig)
    th = lwork.tile([88, RQ], F32, tag=f"th{hf}",
                    name=f"th{lay}_{t}_{hf}")[64:88, :]
    nc.scalar.activation(th, c01, AF.Tanh)
    h = hpool.tile([24, RQ], BF16, tag=f"h{lay}_{hf}", name=f"h{lay}_{t}_{hf}")
    nc.vector.tensor_mul(h, sfi[64:88, :], th)
    st.hprev[hf] = h
    return h


def _attention_group(nc, awork, pv, cg, ub, vcols, dcols, negd, wpv, wpv_d,
                     maskT, pfx, head_group=False):
    """One GRP-chunk group of the masked-softmax attention accumulation.
    head_group: pipeline-head group (nothing queued on DVE yet) — build the
    non-ACT tq chunks on the idle DVE (4x tensor_scalar, and they feed the
    DVE TT with no cross-engine hop) instead of on Pool."""
    tq = awork.tile([128, GRP, R], BF16, tag="tq", name=f"tq_{pfx}_{cg}")
    for q in range(GRP):
        c = cg * GRP + q
        if q in Q_ACT:
            nc.scalar.activation(tq[:, q, :], ub, AF.Relu,
                                 scale=vcols[:, c, :],
                                 bias=negd[:, c, :])
        else:
            eng = nc.vector if (head_group or q in Q_DVE) else nc.gpsimd
            eng.tensor_scalar(tq[:, q, :], ub, scalar1=vcols[:, c, :],
                              scalar2=dcols[:, c, :],
                              op0=OP.mult, op1=OP.max)
    e3 = awork.tile([128, GRP, R], BF16, tag="e3", name=f"e3_{pfx}_{cg}")
    nc.vector.tensor_mul(e3, tq, maskT[:, cg * GRP:(cg + 1) * GRP, :])
    last = cg == NGRP - 1
    for q in range(GRP):
        c = cg * GRP + q
        for s in range(NSUB):
            nc.tensor.matmul(pv[:, s, :], e3[:, q, 128 * s:128 * (s + 1)],
                             wpv[:, c, :], start=(c == 0),
                             stop=(last and q == GRP - 1))
        if q in Q_ACT:
            # mask*D_j correction for the relu form
            for s in range(NSUB):
                nc.tensor.matmul(pv[:, s, :],
                                 maskT[:, c, 128 * s:128 * (s + 1)],
                                 wpv_d[:, cg * len(Q_ACT) + Q_ACT.index(q), :],
                                 start=False, stop=False)


def _elu_into(nc, awork, dst, z, pfx):
    """dst = elu(z) = min(exp(z),1)-1 + max(z,0), elementwise."""
    ez = awork.tile(list(z.shape), F32, tag="elu_ez", name=f"ez_{pfx}")
    nc.scalar.activation(ez, z, AF.Exp)
    nc.gpsimd.tensor_scalar(ez, ez, scalar1=1.0, scalar2=-1.0,
                            op0=OP.min, op1=OP.add)
    zr = awork.tile(list(z.shape), F32, tag="elu_zr", name=f"zr_{pfx}")
    nc.scalar.activation(zr, z, AF.Relu)
    nc.gpsimd.tensor_add(dst, ez, zr)


def _ubcast(nc, psf1, awork, ubpool, ones1, coefT, feats, nk, scale, pfx):
    """u = exp(scale * (coefT.T @ feats)) broadcast over partitions."""
    pf1 = psf1.tile([1, R], F32, tag="f1r", name=f"pf1_{pfx}")
    if nk == 1:
        nc.tensor.matmul(pf1, coefT, feats, start=True, stop=True)
    else:
        for fc in range(nk):
            nc.tensor.matmul(pf1, coefT[:, fc, :], feats[:, fc, :],
                             start=(fc == 0), stop=(fc == nk - 1))
    f1row = awork.tile([1, R], BF16, tag="f1row", name=f"f1row_{pfx}")
    nc.scalar.copy(f1row, pf1)
    pf1b = psf1.tile([128, R], F32, tag="f1r", name=f"pf1b_{pfx}")
    nc.tensor.matmul(pf1b, ones1, f1row, start=True, stop=True)
    ub = ubpool.tile([128, R], BF16, tag=f"ub_{pfx}", name=f"ub_{pfx}")
    nc.scalar.activation(ub, pf1b, AF.Exp, scale=scale)
    return ub


def _build_program():
    nc = bass.Bass()

    xp = nc.dram_tensor("xp", [2 * NIN, SEQ, RH], BF16, kind="ExternalInput")
    adjTb = nc.dram_tensor("adjTb", [N, R], BF16, kind="ExternalInput")
    lwts = nc.dram_tensor("lwts", [128, 128], F32, kind="ExternalInput")
    bds = nc.dram_tensor("bds", [128, 2], F32, kind="ExternalInput")
    wcat = nc.dram_tensor("wcat", [NHEADS, FEAT, NHID + 2], FP8,
                          kind="ExternalInput")
    wocat = nc.dram_tensor("wocat", [NHEADS * NHID, NCLASS + 2], BF16,
                           kind="ExternalInput")
    outb = nc.dram_tensor("outb", [R, NCLASS], F32, kind="ExternalOutput")

    with tile.TileContext(nc) as tc:
        with tc.tile_pool(name="cst", bufs=1) as cst, \
             tc.tile_pool(name="dram", bufs=1, space="DRAM") as dram:

            ident = cst.tile([128, 128], BF16)
            make_identity(nc, ident)
            ones1 = cst.tile([1, 128], BF16)
            nc.vector.memset(ones1, 1.0)
            maskT = cst.tile([128, NJC, R], BF16)
            hT_own = cst.tile([FEAT, R], FP8)
            # gathered features split in two tiles so group-0 prep only
            # waits on the first (smaller) load
            hT_a = cst.tile([FEAT, 2, R], FP8)
            hT_b = cst.tile([FEAT, NCORES - 2, R], FP8)
            wcsb = cst.tile([FEAT, NHEADS, NHID + 2], FP8)
            wocsb = cst.tile([128, NSUB, NCLASS + 2], BF16)
            dumA = cst.tile([1, 2], F32)
            dumB = cst.tile([1, 2], F32)

            g1in = dram.tile([FEAT, R], FP8)
            g1out = dram.tile([NCORES * FEAT, R], FP8, addr_space="Shared")
            g2in = dram.tile([R, GOUT], BF16)
            g2out = dram.tile([N, GOUT], BF16, addr_space="Shared")

            # ======== Phase 1: LSTM (own nodes, 2 groups packed) ===========
            with tc.tile_pool(name="p1", bufs=1) as p1, \
                 tc.tile_pool(name="psg", bufs=4, space="PSUM") as psg, \
                 tc.tile_pool(name="hpool0", bufs=SEQ) as hpool0, \
                 tc.tile_pool(name="hpool1", bufs=3) as hpool1, \
                 tc.tile_pool(name="lwork", bufs=6) as lwork:

                xp_sb = p1.tile([2 * NIN, SEQ, RH], BF16)
                nc.sync.dma_start(out=xp_sb, in_=xp[:])
                lw = p1.tile([128, 128], F32)
                nc.sync.dma_start(out=lw, in_=lwts[:])
                bt = p1.tile([128, 2], F32)
                nc.sync.dma_start(out=bt, in_=bds[:])
                # head weights + output weights: single DMAs, early
                nc.sync.dma_start(out=wcsb,
                                  in_=wcat[:].rearrange("h f c -> f h c"))
                nc.sync.dma_start(
                    out=wocsb, in_=wocat.rearrange("(c p) f -> p c f", p=128))
                # mask loads at t=0: SP is otherwise idle during the LSTM
                adjTr = adjTb[:].rearrange("(c p) r -> p c r", p=128)
                for mg in range(4):
                    nc.sync.dma_start(out=maskT[:, 8 * mg:8 * (mg + 1), :],
                                      in_=adjTr[:, 8 * mg:8 * (mg + 1), :])

                # preload Sigmoid/Tanh activation tables during input DMAs
                nc.vector.memset(dumA, 0.0)
                nc.scalar.activation(dumB, dumA, AF.Sigmoid)
                nc.scalar.activation(dumB, dumA, AF.Tanh)

                w0 = p1.tile([2 * NIN, 128], BF16)
                w0h = p1.tile([24, 128], BF16)
                w1 = p1.tile([24, 128], BF16)
                w1h = p1.tile([24, 128], BF16)
                nc.vector.tensor_copy(w0, lw[0:2 * NIN, :])
                nc.vector.tensor_copy(w0h, lw[32:56, :])
                nc.vector.tensor_copy(w1, lw[64:88, :])
                nc.vector.tensor_copy(w1h, lw[96:120, :])
                b0 = bt[:, 0:1]
                b1 = bt[:, 1:2]

                st0 = _LstmState(0, p1, w0, w0h, b0,
                                 lambda t, hf: xp_sb[:, t, RQ * hf:RQ * (hf + 1)])
                h0s = [[], []]
                st1 = _LstmState(1, p1, w1, w1h, b1,
                                 lambda t, hf: h0s[hf][t])

                # software-pipeline: layer 1 runs one step behind layer 0;
                # the two free-halves are independent chains, so four chains
                # interleave on every engine queue. Layer-1 h accumulates in
                # SBUF (free-dim placement keeps partition bases legal) so
                # only 4 batched DMAs publish g1in at the end.
                hacc = [p1.tile([24, SEQ, RQ], FP8, name=f"hacc{hf}")
                        for hf in (0, 1)]
                # g1in[12t+l, 128*(2g+hf)+c] = hacc[hf][12g+l, t, c]
                g1v = g1in[:].rearrange("(t l) (b c) -> l t b c", l=LH, b=4)
                for slot in range(SEQ + 1):
                    for hf in (0, 1):
                        if slot < SEQ:
                            h0s[hf].append(_lstm_step(nc, st0, slot, hf, psg,
                                                      lwork, hpool0))
                    for hf in (0, 1):
                        if slot >= 1:
                            t = slot - 1
                            h1 = _lstm_step(nc, st1, t, hf, psg, lwork,
                                            hpool1)
                            nc.gpsimd.tensor_copy(hacc[hf][:, t, :], h1)
                    if slot == SEQ - 2:
                        # steps 0..5 are final: publish them while steps 6-7
                        # compute, leaving only tiny stores at the end
                        for g in (0, 1):
                            for hf in (0, 1):
                                nc.sync.dma_start(
                                    out=g1v[:, 0:SEQ - 2, 2 * g + hf, :],
                                    in_=hacc[hf][LH * g:LH * (g + 1),
                                                 0:SEQ - 2, :])
                # final 2 steps: 2 stores on SP + 2 on Act so they drain in
                # parallel right as the last h lands
                for gi, (g, hf) in enumerate(((0, 0), (0, 1), (1, 0), (1, 1))):
                    eng = nc.sync if gi < 2 else nc.scalar
                    eng.dma_start(
                        out=g1v[:, SEQ - 2:SEQ, 2 * g + hf, :],
                        in_=hacc[hf][LH * g:LH * (g + 1), SEQ - 2:SEQ, :])
                nc.gpsimd.collective_compute(
                    "AllGather", OP.bypass,
                    replica_groups=[list(range(NCORES))],
                    ins=[g1in[:].opt()],
                    outs=[g1out[:].opt()])
                nc.sync.dma_start(out=hT_own, in_=g1in)

            # ======== Phase 2: 8 GAT heads + output GAT layer ===============
            with tc.tile_pool(name="att", bufs=1) as att, \
                 tc.tile_pool(name="pstr", bufs=1, space="PSUM") as pstr, \
                 tc.tile_pool(name="pswh", bufs=3, space="PSUM") as pswh, \
                 tc.tile_pool(name="psf1", bufs=1, space="PSUM") as psf1, \
                 tc.tile_pool(name="psout", bufs=1, space="PSUM") as psout, \
                 tc.tile_pool(name="pspv", bufs=2, space="PSUM") as pspv, \
                 tc.tile_pool(name="hw", bufs=4) as hw, \
                 tc.tile_pool(name="awork", bufs=4) as awork:

                hcat = att.tile([128, NSUB, NHEADS * NHID], BF16)
                hcatT = att.tile([128, NSUB, R], BF16)

                # preload Exp/Ln/Relu tables + u for all 8 heads (overlaps
                # the g1 AllGather latency: needs only hT_own)
                nc.scalar.activation(dumB, dumA, AF.Exp)
                nc.scalar.activation(dumB, dumA, AF.Ln)
                nc.scalar.activation(dumB, dumA, AF.Relu)
                ubs = []
                for h in range(NHEADS):
                    ubs.append(_ubcast(nc, psf1, awork, att, ones1,
                                       wcsb[:, h, NHID:NHID + 1], hT_own, 1,
                                       1.0 - ALPHA, f"h{h}"))

                # gathered features: 2 DMAs so head-0 prep starts early
                g1r = g1out[:].rearrange("(b f) r -> f b r", f=FEAT)
                nc.sync.dma_start(out=hT_a, in_=g1r[:, 0:2, :])
                nc.sync.dma_start(out=hT_b, in_=g1r[:, 2:NCORES, :])

                def _head_prep_start(h):
                    whpv = hw.tile([128, NJC, NHID + 1], BF16, tag="whpv",
                                   name=f"whpv{h}")
                    nc.vector.memset(whpv[:, :, NHID:NHID + 1], 1.0)
                    f2cols = hw.tile([128, NJC, 1], F32, tag="f2cols",
                                     name=f"f2cols{h}")
                    vcols = hw.tile([128, NJC, 1], F32, tag="vcols",
                                    name=f"vcols{h}")
                    dcols = hw.tile([128, NJC, 1], F32, tag="dcols",
                                    name=f"dcols{h}")
                    negd = hw.tile([128, NJC, 1], F32, tag="negd",
                                   name=f"negd{h}")
                    wpvd = hw.tile([128, NGRP * len(Q_ACT), NHID + 1], BF16,
                                   tag="wpvd", name=f"wpvd{h}")
                    return [whpv, f2cols, vcols, dcols, negd, wpvd, None,
                            None]

                def _head_prep_piece(h, st, gq):
                    # Wh (+f2) for 4 chunks; small pieces keep the in-order
                    # ACT queue smooth so attention relu-P1s are not stalled
                    # behind a prep burst
                    whpv, f2cols = st[0], st[1]
                    pw4 = pswh.tile([128, 4, NHID + 2], F32, tag="wh",
                                    name=f"pw{h}_{gq}")
                    for k in range(4):
                        c = 4 * gq + k
                        blk, co = c // 4, 128 * (c % 4)
                        src = (hT_a[:, blk, co:co + 128] if blk < 2
                               else hT_b[:, blk - 2, co:co + 128])
                        nc.tensor.matmul(pw4[:, k, :], src,
                                         wcsb[:, h, :], start=True, stop=True)
                    nc.scalar.copy(whpv[:, 4 * gq:4 * (gq + 1), 0:NHID],
                                   pw4[:, :, 0:NHID])
                    nc.scalar.copy(f2cols[:, 4 * gq:4 * (gq + 1), :],
                                   pw4[:, :, NHID + 1:NHID + 2])

                def _factors_group(h, st, cg):
                    # V/D/negD for chunk-group cg (pieces 2cg, 2cg+1); the
                    # wpvd scalings are emitted just-in-time at the consuming
                    # attention group so they never head-of-line block DVE
                    whpv, f2cols, vcols, dcols, negd, wpvd = st[:6]
                    cs = slice(GRP * cg, GRP * (cg + 1))
                    nc.scalar.activation(vcols[:, cs, :], f2cols[:, cs, :],
                                         AF.Exp)
                    nc.scalar.activation(dcols[:, cs, :], f2cols[:, cs, :],
                                         AF.Exp, scale=ALPHA)
                    nc.gpsimd.tensor_scalar(negd[:, cs, :], dcols[:, cs, :],
                                            scalar1=-1.0, scalar2=None,
                                            op0=OP.mult)

                def _wpvd_jit(st, cg):
                    whpv, _, _, dcols, _, wpvd = st[:6]
                    for qi, q in enumerate(Q_ACT):
                        c = cg * GRP + q
                        nc.vector.tensor_scalar_mul(
                            wpvd[:, cg * len(Q_ACT) + qi, :],
                            whpv[:, c, :], dcols[:, c, :])

                def _prep_task(h, st, j):
                    _head_prep_piece(h, st, 2 * j)
                    _head_prep_piece(h, st, 2 * j + 1)
                    _factors_group(h, st, j)

                def _prep_pe(h, st, j):
                    # PE half of a prep task: Wh matmuls for pieces 2j, 2j+1
                    # (slots into the PE idle window under the mask-mul TT)
                    for gq in (2 * j, 2 * j + 1):
                        pw4 = pswh.tile([128, 4, NHID + 2], F32, tag="wh",
                                        name=f"pw{h}_{gq}")
                        for k in range(4):
                            c = 4 * gq + k
                            blk, co = c // 4, 128 * (c % 4)
                            src = (hT_a[:, blk, co:co + 128] if blk < 2
                                   else hT_b[:, blk - 2, co:co + 128])
                            nc.tensor.matmul(pw4[:, k, :], src,
                                             wcsb[:, h, :], start=True,
                                             stop=True)
                        st[6 + gq % 2] = pw4

                def _prep_act(h, st, j):
                    # ACT half: PSUM->SBUF copies + factor exps
                    whpv, f2cols = st[0], st[1]
                    for gq in (2 * j, 2 * j + 1):
                        pw4 = st[6 + gq % 2]
                        nc.scalar.copy(whpv[:, 4 * gq:4 * (gq + 1), 0:NHID],
                                       pw4[:, :, 0:NHID])
                        nc.scalar.copy(f2cols[:, 4 * gq:4 * (gq + 1), :],
                                       pw4[:, :, NHID + 1:NHID + 2])
                    _factors_group(h, st, j)

                def _head_post(h, pv):
                    zall = awork.tile([128, NSUB, NHID], F32, tag="zall",
                                      name=f"zall{h}")
                    for s in range(NSUB):
                        rcp = awork.tile([128, 1], F32, tag="rcp",
                                         name=f"rcp{h}_{s}")
                        nc.vector.reciprocal(rcp, pv[:, s, NHID:NHID + 1])
                        nc.vector.tensor_scalar_mul(zall[:, s, :],
                                                    pv[:, s, 0:NHID], rcp)
                    _elu_into(nc, awork, hcat[:, :, NHID * h:NHID * (h + 1)],
                              zall, f"h{h}")

                # output-layer projection: per-pair PSUM groups (a PE
                # accumulation group must not stay open across interleaved
                # transposes), summed into an SBUF accumulator
                pwacc = att.tile([128, NSUB, NCLASS + 2], F32)
                pwos = {}

                def _transpose_piece(hp, s):
                    ptr = pstr.tile([128, 128], BF16, tag="tr",
                                    name=f"trp{hp}_{s}")
                    nc.tensor.transpose(
                        ptr, hcat[:, s, 128 * hp:128 * (hp + 1)], ident)
                    nc.scalar.copy(hcatT[:, hp, 128 * s:128 * (s + 1)], ptr)
                    if hp not in pwos:
                        pwos[hp] = psout.tile([128, NSUB, NCLASS + 2], F32,
                                              tag="pwo", name=f"pwo{hp}")
                    nc.tensor.matmul(pwos[hp][:, s, :],
                                     hcatT[:, hp, 128 * s:128 * (s + 1)],
                                     wocsb[:, hp, :], start=True, stop=True)

                def _pwo_accum(hp):
                    if hp == 0:
                        nc.vector.tensor_copy(pwacc, pwos[hp])
                    else:
                        nc.vector.tensor_add(pwacc, pwacc, pwos[hp])
                    del pwos[hp]

                def _prep_for(h, j, fn):
                    if h >= NHEADS or j > 3:
                        return
                    if h not in sts:
                        sts[h] = _head_prep_start(h)
                    fn(h, sts[h], j)

                sts = {0: _head_prep_start(0), 1: _head_prep_start(1)}
                _prep_task(0, sts[0], 0)
                _prep_task(1, sts[1], 0)
                _prep_task(0, sts[0], 1)
                _prep_task(1, sts[1], 1)

                # prep emission: a full pair of lookahead. PE halves of
                # pair hp+1's tasks go BEFORE the slot's attention group
                # (the PE is idle under the mask-mul TT); ACT halves go
                # after the first head's group so they never delay the
                # current slot's relu builds. Pair 0's own heads are
                # just-in-time whole tasks.
                for hp in range(NHEADS // 2):
                    ha, hb = 2 * hp, 2 * hp + 1
                    pa = sts[ha]
                    pb = sts[hb]
                    pva = pspv.tile([128, NSUB, NHID + 1], F32, tag="pv",
                                    name=f"pv_h{ha}")
                    pvb = pspv.tile([128, NSUB, NHID + 1], F32, tag="pv",
                                    name=f"pv_h{hb}")
                    for cg in range(NGRP):
                        if hp == 0 and 1 <= cg < 3:
                            _prep_for(ha, cg + 1, _prep_task)
                        _prep_for(ha + 2, cg, _prep_pe)
                        _wpvd_jit(pa, cg)
                        _attention_group(nc, awork, pva, cg, ubs[ha], pa[2],
                                         pa[3], pa[4], pa[0], pa[5], maskT,
                                         f"h{ha}",
                                         head_group=(hp == 0 and cg == 0))
                        _prep_for(ha + 2, cg, _prep_act)
                        if hp == 0 and 1 <= cg < 3:
                            _prep_for(hb, cg + 1, _prep_task)
                        _prep_for(hb + 2, cg, _prep_pe)
                        _wpvd_jit(pb, cg)
                        _attention_group(nc, awork, pvb, cg, ubs[hb], pb[2],
                                         pb[3], pb[4], pb[0], pb[5], maskT,
                                         f"h{hb}")
                        _prep_for(hb + 2, cg, _prep_act)
                        if hp >= 1:
                            # deferred: previous pair's hcatT transpose +
                            # output-projection piece (one sub-block per
                            # slot). Deferring keeps these off the PE queue
                            # head at the pair boundary, where they would
                            # stall the next pair behind the elu chain.
                            _transpose_piece(hp - 1, cg)
                    del sts[ha], sts[hb]
                    _head_post(ha, pva)
                    _head_post(hb, pvb)
                    if hp >= 1:
                        _pwo_accum(hp - 1)

                for s in range(NSUB):
                    _transpose_piece(NHEADS // 2 - 1, s)
                _pwo_accum(NHEADS // 2 - 1)

                # ---- publish output-layer gather payload ----
                # g2 row: [Whout(0:16) | ones(16) | f2o(17)]
                g2stage = awork.tile([128, NSUB, GOUT], BF16, tag="g2stage")
                nc.vector.memset(g2stage[:, :, NCLASS:NCLASS + 1], 1.0)
                nc.scalar.copy(g2stage[:, :, 0:NCLASS], pwacc[:, :, 0:NCLASS])
                nc.scalar.copy(g2stage[:, :, NCLASS + 1:NCLASS + 2],
                               pwacc[:, :, NCLASS + 1:NCLASS + 2])
                nc.sync.dma_start(
                    out=g2in[:].rearrange("(c p) f -> p c f", p=128),
                    in_=g2stage)

                ub_o = _ubcast(nc, psf1, awork, att, ones1,
                               wocsb[:, :, NCLASS:NCLASS + 1], hcatT, NSUB,
                               1.0 - ALPHA, "o")

                nc.gpsimd.collective_compute(
                    "AllGather", OP.bypass,
                    replica_groups=[list(range(NCORES))],
                    ins=[g2in[:].opt()], outs=[g2out[:].opt()])

                # ---- output attention (pipelined per group) ----
                g2full = hw.tile([128, NJC, GOUT], BF16, tag="g2full",
                                 name="g2full")
                vocols = hw.tile([128, NJC, 1], F32, tag="vcols",
                                 name="vocols")
                docols = hw.tile([128, NJC, 1], F32, tag="dcols",
                                 name="docols")
                negdo = hw.tile([128, NJC, 1], F32, tag="negd", name="negdo")
                wpvdo = hw.tile([128, NGRP * len(Q_ACT), NCLASS + 1],
                                BF16, tag="wpvd", name="wpvdo")
                pvo = pspv.tile([128, NSUB, NCLASS + 1], F32, tag="pv",
                                name="pv_o")
                g2r = g2out[:].rearrange("(c p) f -> p c f", p=128)
                for cg in range(NGRP):
                    cs = slice(GRP * cg, GRP * (cg + 1))
                    nc.sync.dma_start(out=g2full[:, cs, :], in_=g2r[:, cs, :])
                    nc.scalar.activation(vocols[:, cs, :],
                                         g2full[:, cs, NCLASS + 1:NCLASS + 2],
                                         AF.Exp)
                    nc.scalar.activation(docols[:, cs, :],
                                         g2full[:, cs, NCLASS + 1:NCLASS + 2],
                                         AF.Exp, scale=ALPHA)
                    nc.gpsimd.tensor_scalar(negdo[:, cs, :], docols[:, cs, :],
                                            scalar1=-1.0, scalar2=None,
                                            op0=OP.mult)
                    for qi, q in enumerate(Q_ACT):
                        c = cg * GRP + q
                        nc.vector.tensor_scalar_mul(
                            wpvdo[:, cg * len(Q_ACT) + qi, :],
                            g2full[:, c, 0:NCLASS + 1], docols[:, c, :])
                    _attention_group(nc, awork, pvo, cg, ub_o, vocols,
                                     docols, negdo,
                                     g2full[:, :, 0:NCLASS + 1], wpvdo,
                                     maskT, "o", head_group=(cg == 0))
                zoall = awork.tile([128, NSUB, NCLASS], F32, tag="zoall")
                for s in range(NSUB):
                    rcp = awork.tile([128, 1], F32, tag="rcp", name=f"rcpo{s}")
                    nc.vector.reciprocal(rcp, pvo[:, s, NCLASS:NCLASS + 1])
                    nc.vector.tensor_scalar_mul(zoall[:, s, :],
                                                pvo[:, s, 0:NCLASS], rcp)
                ziall = awork.tile([128, NSUB, NCLASS], F32, tag="ziall")
                _elu_into(nc, awork, ziall, zoall, "oall")
                ls = awork.tile([128, NSUB, NCLASS], F32, tag="ls", name="ls")
                for s in range(NSUB):
                    zi = ziall[:, s, :]
                    edump = awork.tile([128, NCLASS], F32, tag="edump",
                                       name=f"ed{s}")
                    ssum = awork.tile([128, 1], F32, tag="ssum", name=f"ss{s}")
                    nc.scalar.activation(edump, zi, AF.Exp, accum_out=ssum)
                    lns = awork.tile([128, 1], F32, tag="lns", name=f"ln{s}")
                    nc.scalar.activation(lns, ssum, AF.Ln)
                    nc.vector.tensor_scalar(ls[:, s, :], zi, scalar1=lns,
                                            scalar2=None, op0=OP.subtract)
                nc.sync.dma_start(
                    out=outb[:].rearrange("(c p) f -> p c f", p=128), in_=ls)

    _split_sync_waits(nc)
    return nc


_NC_CACHE = None

_GATE_BASE = {0: 0, 1: 32, 2: 96, 3: 64}  # pytorch i,f,g,o -> partition base


def _pack_wih(w):
    """[4H, in] -> block-diag packed [2*in, 128] bf16: group0 inputs at rows
    0:in -> gate cols base+0:12; group1 at rows in:2*in -> base+12:24."""
    w = np.asarray(w, np.float32)
    nin = w.shape[1]
    out = np.zeros((2 * nin, 128), np.float32)
    for k in range(4):
        base = _GATE_BASE[k]
        blk = w[LH * k:LH * (k + 1), :].T  # [in, 12]
        out[0:nin, base:base + LH] = blk
        out[nin:2 * nin, base + LH:base + 2 * LH] = blk
    return out.astype(ml_dtypes.bfloat16)


def _pack_bias(ba, bb):
    b = np.asarray(ba, np.float32) + np.asarray(bb, np.float32)
    out = np.zeros((128, 1), np.float32)
    for k in range(4):
        base = _GATE_BASE[k]
        out[base:base + LH, 0] = b[LH * k:LH * (k + 1)]
        out[base + LH:base + 2 * LH, 0] = b[LH * k:LH * (k + 1)]
    return out


def kernel(x, adj, Wih0, Whh0, bih0, bhh0, Wih1, Whh1, bih1, bhh1,
           W_heads, a_heads, W_out, a_out):
    global _NC_CACHE
    if _NC_CACHE is None:
        _NC_CACHE = _build_program()
    nc = _NC_CACHE

    x = np.asarray(x, np.float32)
    adj = np.asarray(adj, np.int32)
    W_heads = np.asarray(W_heads, np.float32)
    a_heads = np.asarray(a_heads, np.float32)
    W_out = np.asarray(W_out, np.float32)
    a_out = np.asarray(a_out, np.float32)

    wcat = np.concatenate(
        [W_heads,
         W_heads @ a_heads[:, :NHID, :],
         W_heads @ a_heads[:, NHID:, :]],
        axis=2).astype(ml_dtypes.float8_e4m3fn)
    # f1 coef at col 16 (used for ub_o), f2 coef at col 17: pwo then carries
    # f2o at col 17 which g2stage forwards as gather column 17
    wocat = np.concatenate(
        [W_out, W_out @ a_out[:NCLASS], W_out @ a_out[NCLASS:]],
        axis=1).astype(ml_dtypes.bfloat16)

    lwts = np.zeros((128, 128), np.float32)
    lwts[0:2 * NIN] = _pack_wih(Wih0).astype(np.float32)
    lwts[32:56] = _pack_wih(Whh0).astype(np.float32)
    lwts[64:88] = _pack_wih(Wih1).astype(np.float32)
    lwts[96:120] = _pack_wih(Whh1).astype(np.float32)
    bds = np.concatenate([_pack_bias(bih0, bhh0),
                          _pack_bias(bih1, bhh1)], axis=1)
    common = {
        "lwts": lwts,
        "bds": np.ascontiguousarray(bds.astype(np.float32)),
        "wcat": np.ascontiguousarray(wcat),
        "wocat": np.ascontiguousarray(wocat),
    }
    adjT = adj.T.astype(ml_dtypes.bfloat16)  # [N(cols j), N(rows i)]
    in_maps = []
    for i in range(NCORES):
        blk = slice(R * i, R * (i + 1))
        xb = x[blk]  # [512, 8, 2]
        xpk = np.concatenate(
            [xb[0:RH].transpose(2, 1, 0), xb[RH:R].transpose(2, 1, 0)],
            axis=0)  # [4, 8, 256]
        in_maps.append({
            "xp": np.ascontiguousarray(xpk).astype(ml_dtypes.bfloat16),
            "adjTb": np.ascontiguousarray(adjT[:, blk]),
            **common,
        })

    res = run_bass_kernel_spmd(nc, in_maps, list(range(NCORES)), **_RUN_KWARGS)
    global _LAST_RESULTS
    _LAST_RESULTS = res
    return np.concatenate([res.results[i]["outb"] for i in range(NCORES)],
                          axis=0)


_RUN_KWARGS = {}
_LAST_RESULTS = None


# revision 41
# speedup vs baseline: 1.0067x; 1.0019x over previous
"""Trainium2 Bass kernel for nn_GAT_with_LSTM (2-layer LSTM -> 8-head GAT -> GAT out).

Sharding: node/row dimension split across 8 cores (512 rows each).

Key algebraic restructuring of the GAT attention (vs. direct
exp(leakyrelu(f1+f2)) evaluation): with z = f1_i + f2_j and slope a,
    leakyrelu(z) = max(z, a*z)  =>  e = exp(lrelu(z)) = max(exp(z), exp(a*z)).
Softmax rows are invariant to any per-row (i) factor, so divide by
exp(a*f1_i):
    e'_ij = max(u_i * V_j, D_j),   u = exp((1-a)*f1), V = exp(f2), D = exp(a*f2).
This removes every full-matrix transcendental: exp() runs only on the rank-1
factors. Per 128-column chunk the e-row-block is built one of three ways,
chosen to balance engines:
  - DVE:  t = (ub * V_j) max D_j      (dual-op tensor_scalar, bf16 4x mode)
  - Pool: same op at 1x
  - ACT:  r = relu(V_j * ub - D_j)    (per-partition scale/bias APs); the
          missing mask*D_j term is added back on the PE as
          maskT_chunk @ (D (*) wpv), exact since mask is 0/1:
          mask*max(uV,D) = mask*r + mask*D.
The mask multiply runs as two tensor_tensors (DVE chunks 0:4 at bf16 2x,
Pool chunks 4:8), and the PE accumulates numerator and denominator together
(wpv's last column is ones).

The mask arrives host-side pre-transposed and pre-cast to bf16 (adj[blk].T),
loaded at t=0 while the LSTM runs. The LSTM packs two 256-node groups into
the partition dim with block-diagonal host-packed weights (one sigmoid op
covers i+f+o), runs bf16 matmuls, and software-pipelines layer 1 one step
behind layer 0. LSTM h outputs accumulate in SBUF so 4 batched DMAs
publish g1in for the AllGather (the baseline used 32 serialized stores).
The LSTM feature AllGather ships fp8e4 (f1/f2 logit noise ~0.05 and Wh value
noise average out across the ~2048-wide attention sums).

Activation tables (Sigmoid/Tanh at t=0; Exp/Ln/Relu during the g1 gather)
are preloaded with dummy ops so table loads stay off the critical path.

Head prep (Wh + f2 factors) is pipelined at attention-group granularity:
heads 0/1 prep interleaves with their own attention groups right after the
gather lands; later heads prep 1-2 groups ahead inside the pair loop. The
output-layer projection (pwo) and its gather payload accumulate
incrementally per head-pair on the PE.

Softmax max-subtraction is skipped: attention logits are O(1) (0.1-scale
Xavier weights), exp cannot overflow, softmax is shift-invariant.
"""

import json

import numpy as np
import ml_dtypes

import bass_rust
import concourse.bass as bass
import concourse.tile as tile
from concourse import mybir
from concourse.bass_utils import run_bass_kernel_spmd
from concourse.masks import make_identity

F32 = mybir.dt.float32
BF16 = mybir.dt.bfloat16
FP8 = mybir.dt.float8e4
I32 = mybir.dt.int32
AF = mybir.ActivationFunctionType
OP = mybir.AluOpType

NCORES = 8
N = 4096
R = N // NCORES          # 512 rows per core
SEQ, NIN, LH = 8, 2, 12
FEAT = SEQ * LH          # 96
NHID, NHEADS, NCLASS = 64, 8, 16
ALPHA = 0.2
NJC = N // 128           # 32 j-chunks
NSUB = R // 128          # 4 row sub-blocks per core
GRP = 8                  # j-chunks per group
RH = R // 2              # 256-node half (LSTM partition packing)
NGRP = NJC // GRP        # 4 groups
GOUT = NCLASS + 2        # gathered g2 row: [Whout(16) | ones | f2o]

# chunk-q assignment within each GRP of 8 chunks. The wide mask-mul TT runs
# solo on DVE (one instruction at the bf16 2x rate); splitting it or moving
# tq builds onto DVE loses to per-instruction overhead.
Q_ACT = (5, 6, 7)        # tq via ACT relu (needs mask*D PE correction)
Q_DVE = ()               # remaining tq on Pool


def _split_sync_waits(nc, max_waits=1):
    """This walrus build rejects >1 sync wait per TPB_CTRL instruction
    ("Too many sync wait commands"). Move excess waits onto NoOps inserted
    just before; same-engine program order preserves the semantics."""
    m = json.loads(bass_rust.module_to_json_string(nc.m))
    ctr = 0
    for fn in m["functions"]:
        for bb in fn["blocks"]:
            out = []
            for inst in bb["instructions"]:
                si = inst.get("sync_info")
                ow = (si or {}).get("on_wait") or []
                if len(ow) > max_waits:
                    excess, keep = ow[:-max_waits], ow[-max_waits:]
                    for i in range(0, len(excess), max_waits):
                        ctr += 1
                        out.append({
                            "engine": inst["engine"], "ins": [], "outs": [],
                            "name": f"wsplit-{ctr}", "opcode": "NoOp",
                            "sync_info": {"on_update": [],
                                          "on_wait": excess[i:i + max_waits]},
                        })
                    si["on_wait"] = keep
                out.append(inst)
            bb["instructions"] = out
    nc.m = bass_rust.module_from_json_bytes(json.dumps(m).encode())


RQ = RH // 2  # 128-node quarter: free-dim half of a packed 256 pair


class _LstmState:
    def __init__(self, lay, p1, wih, whh, b, xin):
        self.lay, self.wih, self.whh, self.b, self.xin = lay, wih, whh, b, xin
        # c lives at partition base 32 so TensorTensor partners the f-gate
        # slice (walrus requires equal SBUF base partitions for both inputs)
        self.c01 = [p1.tile([56, RQ], F32, tag=f"c{lay}_{hf}",
                            name=f"c{lay}_{hf}")[32:56, :] for hf in (0, 1)]
        self.hprev = [None, None]


def _lstm_step(nc, st, t, hf, psg, lwork, hpool):
    """One packed LSTM step for free-half hf. Partition layout (two 256-node
    groups packed): i at rows 0:24, f at 32:56, o at 64:88, g at 96:120.
    One sigmoid covers i/f/o. The two free-halves are independent chains, so
    four chains (2 layers x 2 halves) pipeline across the engines."""
    lay = st.lay
    g = psg.tile([128, RQ], F32, tag=f"g{hf}", name=f"g{lay}_{t}_{hf}")
    nc.tensor.matmul(g, st.wih, st.xin(t, hf), start=True, stop=(t == 0))
    if t > 0:
        nc.tensor.matmul(g, st.whh, st.hprev[hf], start=False, stop=True)
    sfi = lwork.tile([88, RQ], F32, tag=f"sfi{hf}", name=f"sfi{lay}_{t}_{hf}")
    nc.scalar.activation(sfi, g[0:88, :], AF.Sigmoid, bias=st.b[0:88, :])
    tg = lwork.tile([24, RQ], F32, tag=f"tg{hf}", name=f"tg{lay}_{t}_{hf}")
    nc.scalar.activation(tg, g[96:120, :], AF.Tanh, bias=st.b[96:120, :])
    c01 = st.c01[hf]
    if t == 0:
        nc.gpsimd.tensor_mul(c01, sfi[0:24, :], tg)
    else:
        # ig at base 32 to partner c01; th at base 64 to partner the o slice.
        # c-mul on Pool, ig/c-add split keeps the DVE (the LSTM-phase
        # bottleneck) down to one op per step.
        ig = lwork.tile([56, RQ], F32, tag=f"ig{hf}",
                        name=f"ig{lay}_{t}_{hf}")[32:56, :]
        nc.gpsimd.tensor_mul(ig, sfi[0:24, :], tg)
        nc.gpsimd.tensor_mul(c01, sfi[32:56, :], c01)
        nc.vector.tensor_add(c01, c01, ig)
    th = lwork.tile([88, RQ], F32, tag=f"th{hf}",
                    name=f"th{lay}_{t}_{hf}")[64:88, :]
    nc.scalar.activation(th, c01, AF.Tanh)
    h = hpool.tile([24, RQ], BF16, tag=f"h{lay}_{hf}", name=f"h{lay}_{t}_{hf}")
    nc.vector.tensor_mul(h, sfi[64:88, :], th)
    st.hprev[hf] = h
    return h


def _attention_group(nc, awork, pv, cg, ub, vcols, dcols, negd, wpv, wpv_d,
                     maskT, pfx, head_group=False):
    """One GRP-chunk group of the masked-softmax attention accumulation.
    head_group: pipeline-head group (nothing queued on DVE yet) — build the
    non-ACT tq chunks on the idle DVE (4x tensor_scalar, and they feed the
    DVE TT with no cross-engine hop) instead of on Pool."""
    tq = awork.tile([128, GRP, R], BF16, tag="tq", name=f"tq_{pfx}_{cg}")
    for q in range(GRP):
        c = cg * GRP + q
        if q in Q_ACT:
            nc.scalar.activation(tq[:, q, :], ub, AF.Relu,
                                 scale=vcols[:, c, :],
                                 bias=negd[:, c, :])
        else:
            eng = nc.vector if (head_group or q in Q_DVE) else nc.gpsimd
            eng.tensor_scalar(tq[:, q, :], ub, scalar1=vcols[:, c, :],
                              scalar2=dcols[:, c, :],
                              op0=OP.mult, op1=OP.max)
    e3 = awork.tile([128, GRP, R], BF16, tag="e3", name=f"e3_{pfx}_{cg}")
    nc.vector.tensor_mul(e3, tq, maskT[:, cg * GRP:(cg + 1) * GRP, :])
    last = cg == NGRP - 1
    for q in range(GRP):
        c = cg * GRP + q
        for s in range(NSUB):
            nc.tensor.matmul(pv[:, s, :], e3[:, q, 128 * s:128 * (s + 1)],
                             wpv[:, c, :], start=(c == 0),
                             stop=(last and q == GRP - 1))
        if q in Q_ACT:
            # mask*D_j correction for the relu form
            for s in range(NSUB):
                nc.tensor.matmul(pv[:, s, :],
                                 maskT[:, c, 128 * s:128 * (s + 1)],
                                 wpv_d[:, cg * len(Q_ACT) + Q_ACT.index(q), :],
                                 start=False, stop=False)


def _elu_into(nc, awork, dst, z, pfx):
    """dst = elu(z) = min(exp(z),1)-1 + max(z,0), elementwise."""
    ez = awork.tile(list(z.shape), F32, tag="elu_ez", name=f"ez_{pfx}")
    nc.scalar.activation(ez, z, AF.Exp)
    nc.gpsimd.tensor_scalar(ez, ez, scalar1=1.0, scalar2=-1.0,
                            op0=OP.min, op1=OP.add)
    zr = awork.tile(list(z.shape), F32, tag="elu_zr", name=f"zr_{pfx}")
    nc.scalar.activation(zr, z, AF.Relu)
    nc.gpsimd.tensor_add(dst, ez, zr)


def _ubcast(nc, psf1, awork, ubpool, ones1, coefT, feats, nk, scale, pfx):
    """u = exp(scale * (coefT.T @ feats)) broadcast over partitions."""
    pf1 = psf1.tile([1, R], F32, tag="f1r", name=f"pf1_{pfx}")
    if nk == 1:
        nc.tensor.matmul(pf1, coefT, feats, start=True, stop=True)
    else:
        for fc in range(nk):
            nc.tensor.matmul(pf1, coefT[:, fc, :], feats[:, fc, :],
                             start=(fc == 0), stop=(fc == nk - 1))
    f1row = awork.tile([1, R], BF16, tag="f1row", name=f"f1row_{pfx}")
    nc.scalar.copy(f1row, pf1)
    pf1b = psf1.tile([128, R], F32, tag="f1r", name=f"pf1b_{pfx}")
    nc.tensor.matmul(pf1b, ones1, f1row, start=True, stop=True)
    ub = ubpool.tile([128, R], BF16, tag=f"ub_{pfx}", name=f"ub_{pfx}")
    nc.scalar.activation(ub, pf1b, AF.Exp, scale=scale)
    return ub


def _build_program():
    nc = bass.Bass()

    xp = nc.dram_tensor("xp", [2 * NIN, SEQ, RH], BF16, kind="ExternalInput")
    adjTb = nc.dram_tensor("adjTb", [N, R], BF16, kind="ExternalInput")
    lwts = nc.dram_tensor("lwts", [128, 128], F32, kind="ExternalInput")
    bds = nc.dram_tensor("bds", [128, 2], F32, kind="ExternalInput")
    wcat = nc.dram_tensor("wcat", [NHEADS, FEAT, NHID + 2], FP8,
                          kind="ExternalInput")
    wocat = nc.dram_tensor("wocat", [NHEADS * NHID, NCLASS + 2], BF16,
                           kind="ExternalInput")
    outb = nc.dram_tensor("outb", [R, NCLASS], F32, kind="ExternalOutput")

    with tile.TileContext(nc) as tc:
        with tc.tile_pool(name="cst", bufs=1) as cst, \
             tc.tile_pool(name="dram", bufs=1, space="DRAM") as dram:

            ident = cst.tile([128, 128], BF16)
            make_identity(nc, ident)
            ones1 = cst.tile([1, 128], BF16)
            nc.vector.memset(ones1, 1.0)
            maskT = cst.tile([128, NJC, R], BF16)
            hT_own = cst.tile([FEAT, R], FP8)
            # gathered features split in two tiles so group-0 prep only
            # waits on the first (smaller) load
            hT_a = cst.tile([FEAT, 2, R], FP8)
            hT_b = cst.tile([FEAT, NCORES - 2, R], FP8)
            wcsb = cst.tile([FEAT, NHEADS, NHID + 2], FP8)
            wocsb = cst.tile([128, NSUB, NCLASS + 2], BF16)
            dumA = cst.tile([1, 2], F32)
            dumB = cst.tile([1, 2], F32)

            g1in = dram.tile([FEAT, R], FP8)
            g1out = dram.tile([NCORES * FEAT, R], FP8, addr_space="Shared")
            g2in = dram.tile([R, GOUT], BF16)
            g2out = dram.tile([N, GOUT], BF16, addr_space="Shared")

            # ======== Phase 1: LSTM (own nodes, 2 groups packed) ===========
            with tc.tile_pool(name="p1", bufs=1) as p1, \
                 tc.tile_pool(name="psg", bufs=4, space="PSUM") as psg, \
                 tc.tile_pool(name="hpool0", bufs=SEQ) as hpool0, \
                 tc.tile_pool(name="hpool1", bufs=3) as hpool1, \
                 tc.tile_pool(name="lwork", bufs=6) as lwork:

                xp_sb = p1.tile([2 * NIN, SEQ, RH], BF16)
                nc.sync.dma_start(out=xp_sb, in_=xp[:])
                lw = p1.tile([128, 128], F32)
                nc.sync.dma_start(out=lw, in_=lwts[:])
                bt = p1.tile([128, 2], F32)
                nc.sync.dma_start(out=bt, in_=bds[:])
                # head weights + output weights: single DMAs, early
                nc.sync.dma_start(out=wcsb,
                                  in_=wcat[:].rearrange("h f c -> f h c"))
                nc.sync.dma_start(
                    out=wocsb, in_=wocat.rearrange("(c p) f -> p c f", p=128))
                # mask loads at t=0: SP is otherwise idle during the LSTM
                adjTr = adjTb[:].rearrange("(c p) r -> p c r", p=128)
                for mg in range(4):
                    nc.sync.dma_start(out=maskT[:, 8 * mg:8 * (mg + 1), :],
                                      in_=adjTr[:, 8 * mg:8 * (mg + 1), :])

                # preload Sigmoid/Tanh activation tables during input DMAs
                nc.vector.memset(dumA, 0.0)
                nc.scalar.activation(dumB, dumA, AF.Sigmoid)
                nc.scalar.activation(dumB, dumA, AF.Tanh)

                w0 = p1.tile([2 * NIN, 128], BF16)
                w0h = p1.tile([24, 128], BF16)
                w1 = p1.tile([24, 128], BF16)
                w1h = p1.tile([24, 128], BF16)
                nc.vector.tensor_copy(w0, lw[0:2 * NIN, :])
                nc.vector.tensor_copy(w0h, lw[32:56, :])
                nc.vector.tensor_copy(w1, lw[64:88, :])
                nc.vector.tensor_copy(w1h, lw[96:120, :])
                b0 = bt[:, 0:1]
                b1 = bt[:, 1:2]

                st0 = _LstmState(0, p1, w0, w0h, b0,
                                 lambda t, hf: xp_sb[:, t, RQ * hf:RQ * (hf + 1)])
                h0s = [[], []]
                st1 = _LstmState(1, p1, w1, w1h, b1,
                                 lambda t, hf: h0s[hf][t])

                # software-pipeline: layer 1 runs one step behind layer 0;
                # the two free-halves are independent chains, so four chains
                # interleave on every engine queue. Layer-1 h accumulates in
                # SBUF (free-dim placement keeps partition bases legal) so
                # only 4 batched DMAs publish g1in at the end.
                hacc = [p1.tile([24, SEQ, RQ], FP8, name=f"hacc{hf}")
                        for hf in (0, 1)]
                # g1in[12t+l, 128*(2g+hf)+c] = hacc[hf][12g+l, t, c]
                g1v = g1in[:].rearrange("(t l) (b c) -> l t b c", l=LH, b=4)
                for slot in range(SEQ + 1):
                    for hf in (0, 1):
                        if slot < SEQ:
                            h0s[hf].append(_lstm_step(nc, st0, slot, hf, psg,
                                                      lwork, hpool0))
                    for hf in (0, 1):
                        if slot >= 1:
                            t = slot - 1
                            h1 = _lstm_step(nc, st1, t, hf, psg, lwork,
                                            hpool1)
                            nc.gpsimd.tensor_copy(hacc[hf][:, t, :], h1)
                    if slot == SEQ - 2:
                        # steps 0..5 are final: publish them while steps 6-7
                        # compute, leaving only tiny stores at the end
                        for g in (0, 1):
                            for hf in (0, 1):
                                nc.sync.dma_start(
                                    out=g1v[:, 0:SEQ - 2, 2 * g + hf, :],
                                    in_=hacc[hf][LH * g:LH * (g + 1),
                                                 0:SEQ - 2, :])
                # final 2 steps: 2 stores on SP + 2 on Act so they drain in
                # parallel right as the last h lands
                for gi, (g, hf) in enumerate(((0, 0), (0, 1), (1, 0), (1, 1))):
                    eng = nc.sync if gi < 2 else nc.scalar
                    eng.dma_start(
                        out=g1v[:, SEQ - 2:SEQ, 2 * g + hf, :],
                        in_=hacc[hf][LH * g:LH * (g + 1), SEQ - 2:SEQ, :])
                nc.gpsimd.collective_compute(
                    "AllGather", OP.bypass,
                    replica_groups=[list(range(NCORES))],
                    ins=[g1in[:].opt()],
                    outs=[g1out[:].opt()])
                nc.sync.dma_start(out=hT_own, in_=g1in)

            # ======== Phase 2: 8 GAT heads + output GAT layer ===============
            with tc.tile_pool(name="att", bufs=1) as att, \
                 tc.tile_pool(name="pstr", bufs=1, space="PSUM") as pstr, \
                 tc.tile_pool(name="pswh", bufs=3, space="PSUM") as pswh, \
                 tc.tile_pool(name="psf1", bufs=1, space="PSUM") as psf1, \
                 tc.tile_pool(name="psout", bufs=1, space="PSUM") as psout, \
                 tc.tile_pool(name="pspv", bufs=2, space="PSUM") as pspv, \
                 tc.tile_pool(name="hw", bufs=4) as hw, \
                 tc.tile_pool(name="awork", bufs=4) as awork:

                hcat = att.tile([128, NSUB, NHEADS * NHID], BF16)
                hcatT = att.tile([128, NSUB, R], BF16)

                # preload Exp/Ln/Relu tables + u for all 8 heads (overlaps
                # the g1 AllGather latency: needs only hT_own)
                nc.scalar.activation(dumB, dumA, AF.Exp)
                nc.scalar.activation(dumB, dumA, AF.Ln)
                nc.scalar.activation(dumB, dumA, AF.Relu)
                ubs = []
                for h in range(NHEADS):
                    ubs.append(_ubcast(nc, psf1, awork, att, ones1,
                                       wcsb[:, h, NHID:NHID + 1], hT_own, 1,
                                       1.0 - ALPHA, f"h{h}"))

                # gathered features: 2 DMAs so head-0 prep starts early
                g1r = g1out[:].rearrange("(b f) r -> f b r", f=FEAT)
                nc.sync.dma_start(out=hT_a, in_=g1r[:, 0:2, :])
                nc.sync.dma_start(out=hT_b, in_=g1r[:, 2:NCORES, :])

                def _head_prep_start(h):
                    whpv = hw.tile([128, NJC, NHID + 1], BF16, tag="whpv",
                                   name=f"whpv{h}")
                    nc.vector.memset(whpv[:, :, NHID:NHID + 1], 1.0)
                    f2cols = hw.tile([128, NJC, 1], F32, tag="f2cols",
                                     name=f"f2cols{h}")
                    vcols = hw.tile([128, NJC, 1], F32, tag="vcols",
                                    name=f"vcols{h}")
                    dcols = hw.tile([128, NJC, 1], F32, tag="dcols",
                                    name=f"dcols{h}")
                    negd = hw.tile([128, NJC, 1], F32, tag="negd",
                                   name=f"negd{h}")
                    wpvd = hw.tile([128, NGRP * len(Q_ACT), NHID + 1], BF16,
                                   tag="wpvd", name=f"wpvd{h}")
                    return [whpv, f2cols, vcols, dcols, negd, wpvd, None,
                            None]

                def _head_prep_piece(h, st, gq):
                    # Wh (+f2) for 4 chunks; small pieces keep the in-order
                    # ACT queue smooth so attention relu-P1s are not stalled
                    # behind a prep burst
                    whpv, f2cols = st[0], st[1]
                    pw4 = pswh.tile([128, 4, NHID + 2], F32, tag="wh",
                                    name=f"pw{h}_{gq}")
                    for k in range(4):
                        c = 4 * gq + k
                        blk, co = c // 4, 128 * (c % 4)
                        src = (hT_a[:, blk, co:co + 128] if blk < 2
                               else hT_b[:, blk - 2, co:co + 128])
                        nc.tensor.matmul(pw4[:, k, :], src,
                                         wcsb[:, h, :], start=True, stop=True)
                    nc.scalar.copy(whpv[:, 4 * gq:4 * (gq + 1), 0:NHID],
                                   pw4[:, :, 0:NHID])
                    nc.scalar.copy(f2cols[:, 4 * gq:4 * (gq + 1), :],
                                   pw4[:, :, NHID + 1:NHID + 2])

                def _factors_group(h, st, cg):
                    # V/D/negD for chunk-group cg (pieces 2cg, 2cg+1); the
                    # wpvd scalings are emitted just-in-time at the consuming
                    # attention group so they never head-of-line block DVE
                    whpv, f2cols, vcols, dcols, negd, wpvd = st[:6]
                    cs = slice(GRP * cg, GRP * (cg + 1))
                    nc.scalar.activation(vcols[:, cs, :], f2cols[:, cs, :],
                                         AF.Exp)
                    nc.scalar.activation(dcols[:, cs, :], f2cols[:, cs, :],
                                         AF.Exp, scale=ALPHA)
                    nc.gpsimd.tensor_scalar(negd[:, cs, :], dcols[:, cs, :],
                                            scalar1=-1.0, scalar2=None,
                                            op0=OP.mult)

                def _wpvd_jit(st, cg):
                    whpv, _, _, dcols, _, wpvd = st[:6]
                    for qi, q in enumerate(Q_ACT):
                        c = cg * GRP + q
                        nc.vector.tensor_scalar_mul(
                            wpvd[:, cg * len(Q_ACT) + qi, :],
                            whpv[:, c, :], dcols[:, c, :])

                def _prep_task(h, st, j):
                    _head_prep_piece(h, st, 2 * j)
                    _head_prep_piece(h, st, 2 * j + 1)
                    _factors_group(h, st, j)

                def _prep_pe(h, st, j):
                    # PE half of a prep task: Wh matmuls for pieces 2j, 2j+1
                    # (slots into the PE idle window under the mask-mul TT)
                    for gq in (2 * j, 2 * j + 1):
                        pw4 = pswh.tile([128, 4, NHID + 2], F32, tag="wh",
                                        name=f"pw{h}_{gq}")
                        for k in range(4):
                            c = 4 * gq + k
                            blk, co = c // 4, 128 * (c % 4)
                            src = (hT_a[:, blk, co:co + 128] if blk < 2
                                   else hT_b[:, blk - 2, co:co + 128])
                            nc.tensor.matmul(pw4[:, k, :], src,
                                             wcsb[:, h, :], start=True,
                                             stop=True)
                        st[6 + gq % 2] = pw4

                def _prep_act(h, st, j):
                    # ACT half: PSUM->SBUF copies + factor exps
                    whpv, f2cols = st[0], st[1]
                    for gq in (2 * j, 2 * j + 1):
                        pw4 = st[6 + gq % 2]
                        nc.scalar.copy(whpv[:, 4 * gq:4 * (gq + 1), 0:NHID],
                                       pw4[:, :, 0:NHID])
                        nc.scalar.copy(f2cols[:, 4 * gq:4 * (gq + 1), :],
                                       pw4[:, :, NHID + 1:NHID + 2])
                    _factors_group(h, st, j)

                def _head_post(h, pv):
                    zall = awork.tile([128, NSUB, NHID], F32, tag="zall",
                                      name=f"zall{h}")
                    for s in range(NSUB):
                        rcp = awork.tile([128, 1], F32, tag="rcp",
                                         name=f"rcp{h}_{s}")
                        nc.vector.reciprocal(rcp, pv[:, s, NHID:NHID + 1])
                        nc.vector.tensor_scalar_mul(zall[:, s, :],
                                                    pv[:, s, 0:NHID], rcp)
                    _elu_into(nc, awork, hcat[:, :, NHID * h:NHID * (h + 1)],
                              zall, f"h{h}")

                # output-layer projection: per-pair PSUM groups (a PE
                # accumulation group must not stay open across interleaved
                # transposes), summed into an SBUF accumulator
                pwacc = att.tile([128, NSUB, NCLASS + 2], F32)
                pwos = {}

                def _transpose_piece(hp, s):
                    ptr = pstr.tile([128, 128], BF16, tag="tr",
                                    name=f"trp{hp}_{s}")
                    nc.tensor.transpose(
                        ptr, hcat[:, s, 128 * hp:128 * (hp + 1)], ident)
                    nc.scalar.copy(hcatT[:, hp, 128 * s:128 * (s + 1)], ptr)
                    if hp not in pwos:
                        pwos[hp] = psout.tile([128, NSUB, NCLASS + 2], F32,
                                              tag="pwo", name=f"pwo{hp}")
                    nc.tensor.matmul(pwos[hp][:, s, :],
                                     hcatT[:, hp, 128 * s:128 * (s + 1)],
                                     wocsb[:, hp, :], start=True, stop=True)

                def _pwo_accum(hp):
                    if hp == 0:
                        nc.vector.tensor_copy(pwacc, pwos[hp])
                    else:
                        nc.vector.tensor_add(pwacc, pwacc, pwos[hp])
                    del pwos[hp]

                def _prep_for(h, j, fn):
                    if h >= NHEADS or j > 3:
                        return
                    if h not in sts:
                        sts[h] = _head_prep_start(h)
                    fn(h, sts[h], j)

                sts = {0: _head_prep_start(0), 1: _head_prep_start(1)}
                _prep_task(0, sts[0], 0)
                _prep_task(1, sts[1], 0)

                # prep emission: a full pair of lookahead. PE halves of
                # pair hp+1's tasks go BEFORE the slot's attention group
                # (the PE is idle under the mask-mul TT); ACT halves go
                # after the first head's group so they never delay the
                # current slot's relu builds. Pair 0's own heads are
                # just-in-time whole tasks.
                for hp in range(NHEADS // 2):
                    ha, hb = 2 * hp, 2 * hp + 1
                    pa = sts[ha]
                    pb = sts[hb]
                    pva = pspv.tile([128, NSUB, NHID + 1], F32, tag="pv",
                                    name=f"pv_h{ha}")
                    pvb = pspv.tile([128, NSUB, NHID + 1], F32, tag="pv",
                                    name=f"pv_h{hb}")
                    for cg in range(NGRP):
                        if hp == 0 and cg < 3:
                            _prep_for(ha, cg + 1, _prep_task)
                        _prep_for(ha + 2, cg, _prep_pe)
                        _wpvd_jit(pa, cg)
                        _attention_group(nc, awork, pva, cg, ubs[ha], pa[2],
                                         pa[3], pa[4], pa[0], pa[5], maskT,
                                         f"h{ha}",
                                         head_group=(hp == 0 and cg == 0))
                        _prep_for(ha + 2, cg, _prep_act)
                        if hp == 0 and cg < 3:
                            _prep_for(hb, cg + 1, _prep_task)
                        _prep_for(hb + 2, cg, _prep_pe)
                        _wpvd_jit(pb, cg)
                        _attention_group(nc, awork, pvb, cg, ubs[hb], pb[2],
                                         pb[3], pb[4], pb[0], pb[5], maskT,
                                         f"h{hb}")
                        _prep_for(hb + 2, cg, _prep_act)
                        if hp >= 1:
                            # deferred: previous pair's hcatT transpose +
                            # output-projection piece (one sub-block per
                            # slot). Deferring keeps these off the PE queue
                            # head at the pair boundary, where they would
                            # stall the next pair behind the elu chain.
                            _transpose_piece(hp - 1, cg)
                    del sts[ha], sts[hb]
                    _head_post(ha, pva)
                    _head_post(hb, pvb)
                    if hp >= 1:
                        _pwo_accum(hp - 1)

                for s in range(NSUB):
                    _transpose_piece(NHEADS // 2 - 1, s)
                _pwo_accum(NHEADS // 2 - 1)

                # ---- publish output-layer gather payload ----
                # g2 row: [Whout(0:16) | ones(16) | f2o(17)]
                g2stage = awork.tile([128, NSUB, GOUT], BF16, tag="g2stage")
                nc.vector.memset(g2stage[:, :, NCLASS:NCLASS + 1], 1.0)
                nc.scalar.copy(g2stage[:, :, 0:NCLASS], pwacc[:, :, 0:NCLASS])
                nc.scalar.copy(g2stage[:, :, NCLASS + 1:NCLASS + 2],
                               pwacc[:, :, NCLASS + 1:NCLASS + 2])
                nc.sync.dma_start(
                    out=g2in[:].rearrange("(c p) f -> p c f", p=128),
                    in_=g2stage)

                ub_o = _ubcast(nc, psf1, awork, att, ones1,
                               wocsb[:, :, NCLASS:NCLASS + 1], hcatT, NSUB,
                               1.0 - ALPHA, "o")

                nc.gpsimd.collective_compute(
                    "AllGather", OP.bypass,
                    replica_groups=[list(range(NCORES))],
                    ins=[g2in[:].opt()], outs=[g2out[:].opt()])

                # ---- output attention (pipelined per group) ----
                g2full = hw.tile([128, NJC, GOUT], BF16, tag="g2full",
                                 name="g2full")
                vocols = hw.tile([128, NJC, 1], F32, tag="vcols",
                                 name="vocols")
                docols = hw.tile([128, NJC, 1], F32, tag="dcols",
                                 name="docols")
                negdo = hw.tile([128, NJC, 1], F32, tag="negd", name="negdo")
                wpvdo = hw.tile([128, NGRP * len(Q_ACT), NCLASS + 1],
                                BF16, tag="wpvd", name="wpvdo")
                pvo = pspv.tile([128, NSUB, NCLASS + 1], F32, tag="pv",
                                name="pv_o")
                g2r = g2out[:].rearrange("(c p) f -> p c f", p=128)
                for cg in range(NGRP):
                    cs = slice(GRP * cg, GRP * (cg + 1))
                    nc.sync.dma_start(out=g2full[:, cs, :], in_=g2r[:, cs, :])
                    nc.scalar.activation(vocols[:, cs, :],
                                         g2full[:, cs, NCLASS + 1:NCLASS + 2],
                                         AF.Exp)
                    nc.scalar.activation(docols[:, cs, :],
                                         g2full[:, cs, NCLASS + 1:NCLASS + 2],
                                         AF.Exp, scale=ALPHA)
                    nc.gpsimd.tensor_scalar(negdo[:, cs, :], docols[:, cs, :],
                                            scalar1=-1.0, scalar2=None,
                                            op0=OP.mult)
                    for qi, q in enumerate(Q_ACT):
                        c = cg * GRP + q
                        nc.vector.tensor_scalar_mul(
                            wpvdo[:, cg * len(Q_ACT) + qi, :],
                            g2full[:, c, 0:NCLASS + 1], docols[:, c, :])
                    _attention_group(nc, awork, pvo, cg, ub_o, vocols,
                                     docols, negdo,
                                     g2full[:, :, 0:NCLASS + 1], wpvdo,
                                     maskT, "o", head_group=(cg == 0))
                zoall = awork.tile([128, NSUB, NCLASS], F32, tag="zoall")
                for s in range(NSUB):
                    rcp = awork.tile([128, 1], F32, tag="rcp", name=f"rcpo{s}")
                    nc.vector.reciprocal(rcp, pvo[:, s, NCLASS:NCLASS + 1])
                    nc.vector.tensor_scalar_mul(zoall[:, s, :],
                                                pvo[:, s, 0:NCLASS], rcp)
                ziall = awork.tile([128, NSUB, NCLASS], F32, tag="ziall")
                _elu_into(nc, awork, ziall, zoall, "oall")
                ls = awork.tile([128, NSUB, NCLASS], F32, tag="ls", name="ls")
                for s in range(NSUB):
                    zi = ziall[:, s, :]
                    edump = awork.tile([128, NCLASS], F32, tag="edump",
                                       name=f"ed{s}")
                    ssum = awork.tile([128, 1], F32, tag="ssum", name=f"ss{s}")
                    nc.scalar.activation(edump, zi, AF.Exp, accum_out=ssum)
                    lns = awork.tile([128, 1], F32, tag="lns", name=f"ln{s}")
                    nc.scalar.activation(lns, ssum, AF.Ln)
                    nc.vector.tensor_scalar(ls[:, s, :], zi, scalar1=lns,
                                            scalar2=None, op0=OP.subtract)
                nc.sync.dma_start(
                    out=outb[:].rearrange("(c p) f -> p c f", p=128), in_=ls)

    _split_sync_waits(nc)
    return nc


_NC_CACHE = None

_GATE_BASE = {0: 0, 1: 32, 2: 96, 3: 64}  # pytorch i,f,g,o -> partition base


def _pack_wih(w):
    """[4H, in] -> block-diag packed [2*in, 128] bf16: group0 inputs at rows
    0:in -> gate cols base+0:12; group1 at rows in:2*in -> base+12:24."""
    w = np.asarray(w, np.float32)
    nin = w.shape[1]
    out = np.zeros((2 * nin, 128), np.float32)
    for k in range(4):
        base = _GATE_BASE[k]
        blk = w[LH * k:LH * (k + 1), :].T  # [in, 12]
        out[0:nin, base:base + LH] = blk
        out[nin:2 * nin, base + LH:base + 2 * LH] = blk
    return out.astype(ml_dtypes.bfloat16)


def _pack_bias(ba, bb):
    b = np.asarray(ba, np.float32) + np.asarray(bb, np.float32)
    out = np.zeros((128, 1), np.float32)
    for k in range(4):
        base = _GATE_BASE[k]
        out[base:base + LH, 0] = b[LH * k:LH * (k + 1)]
        out[base + LH:base + 2 * LH, 0] = b[LH * k:LH * (k + 1)]
    return out


def kernel(x, adj, Wih0, Whh0, bih0, bhh0, Wih1, Whh1, bih1, bhh1,
           W_heads, a_heads, W_out, a_out):
    global _NC_CACHE
    if _NC_CACHE is None:
        _NC_CACHE = _build_program()
    nc = _NC_CACHE

    x = np.asarray(x, np.float32)
    adj = np.asarray(adj, np.int32)
    W_heads = np.asarray(W_heads, np.float32)
    a_heads = np.asarray(a_heads, np.float32)
    W_out = np.asarray(W_out, np.float32)
    a_out = np.asarray(a_out, np.float32)

    wcat = np.concatenate(
        [W_heads,
         W_heads @ a_heads[:, :NHID, :],
         W_heads @ a_heads[:, NHID:, :]],
        axis=2).astype(ml_dtypes.float8_e4m3fn)
    # f1 coef at col 16 (used for ub_o), f2 coef at col 17: pwo then carries
    # f2o at col 17 which g2stage forwards as gather column 17
    wocat = np.concatenate(
        [W_out, W_out @ a_out[:NCLASS], W_out @ a_out[NCLASS:]],
        axis=1).astype(ml_dtypes.bfloat16)

    lwts = np.zeros((128, 128), np.float32)
    lwts[0:2 * NIN] = _pack_wih(Wih0).astype(np.float32)
    lwts[32:56] = _pack_wih(Whh0).astype(np.float32)
    lwts[64:88] = _pack_wih(Wih1).astype(np.float32)
    lwts[96:120] = _pack_wih(Whh1).astype(np.float32)
    bds = np.concatenate([_pack_bias(bih0, bhh0),
                          _pack_bias(bih1, bhh1)], axis=1)
    common = {
        "lwts": lwts,
        "bds": np.ascontiguousarray(bds.astype(np.float32)),
        "wcat": np.ascontiguousarray(wcat),
        "wocat": np.ascontiguousarray(wocat),
    }
    adjT = adj.T.astype(ml_dtypes.bfloat16)  # [N(cols j), N(rows i)]
    in_maps = []
    for i in range(NCORES):
        blk = slice(R * i, R * (i + 1))
        xb = x[blk]  # [512, 8, 2]
        xpk = np.concatenate(
            [xb[0:RH].transpose(2, 1, 0), xb[RH:R].transpose(2, 1, 0)],
            axis=0)  # [4, 8, 256]
        in_maps.append({
            "xp": np.ascontiguousarray(xpk).astype(ml_dtypes.bfloat16),
            "adjTb": np.ascontiguousarray(adjT[:, blk]),
            **common,
        })

    res = run_bass_kernel_spmd(nc, in_maps, list(range(NCORES)), **_RUN_KWARGS)
    global _LAST_RESULTS
    _LAST_RESULTS = res
    return np.concatenate([res.results[i]["outb"] for i in range(NCORES)],
                          axis=0)


_RUN_KWARGS = {}
_LAST_RESULTS = None
